# revision 20
# baseline (speedup 1.0000x reference)
"""Trainium2 Bass kernel for nn_Discriminator_61332132987171 (vq_codebook).

Data-parallel over batch: 8 images per NeuronCore across 8 cores.
All matmuls in float32r (fp32 with 11-bit mantissa, full PE rate at N>=256).

Per-core pipeline:
  block0: im2col conv 3->256 (K=27), conv 256->256 (9-tap accumulation over a
          zero-padded SBUF plane), avgpool2, 1x1-conv shortcut on pooled x
  VQ0..VQ3: T[pix,code] = x.e - |e|^2/2 via matmuls (codes on free dim),
          max-reduce over codes, per-image sums into one PSUM accumulator
  block1: preact block at 16x16 with downsample + 1x1 shortcut
  block2/3: preact blocks at 8x8, image-batched matmuls (3D moving APs)
  head: hf = sum relu(h3) via ACT accum_out; out = hf . (lin/sl + emb[y]/se) + b
  spectral-norm 1/sigma for every weight computed on device and folded into
  the PSUM-eviction activation scale.

Host side only shards/transposes/rounds inputs and reduces per-core histogram
counts into the final perplexity scalar.
"""
import sys

for _p in ("/opt/trn_rl_repo", "/opt/pypackages"):
    if _p not in sys.path:
        sys.path.append(_p)

import numpy as np
import concourse.bass as bass  # noqa: F401
import concourse.mybir as mybir
import concourse.tile as tile
from concourse import bacc
from concourse.bass_utils import run_bass_kernel_spmd

F32 = mybir.dt.float32
F32R = mybir.dt.float32r
AX = mybir.AxisListType
ALU = mybir.AluOpType
ACTF = mybir.ActivationFunctionType

P = 128
NCORES = 8
BPC = 8  # images per core

C0 = 0.5 / (256.0 * 16 * 16)   # quant-loss scale, block0
C1 = 0.5 / (256.0 * 8 * 8)     # blocks 1-3
TAP_ORDER = [4, 0, 1, 2, 3, 5, 6, 7, 8]  # center tap first (partitions 0..2)


def _r32r(x):
    """Round fp32 -> fp32r (11-bit mantissa, RTNE) on host."""
    u = np.ascontiguousarray(x, dtype=np.float32).view(np.uint32)
    u2 = u + 0x7FF + ((u >> 12) & 1)
    return (u2 & 0xFFFFF000).astype(np.uint32).view(np.float32)


class Ctx:
    def __init__(self, nc, tc, pools, inp):
        self.nc = nc
        self.tc = tc
        self.p = pools
        self.inp = inp
        self.uid = 0
        self.qd_first = True

    def tag(self, base):
        self.uid += 1
        return f"{base}{self.uid}"


def emit_bcast11(cx, src11_f32_ap, tagbase):
    """[1,1] f32 AP -> [128,1] f32 SBUF column (K=1 ones matmul broadcast)."""
    nc, p = cx.nc, cx.p
    s_r = p["fix"].tile([1, 1], F32, tag=cx.tag("bc_r"))
    nc.vector.tensor_copy(s_r[:], src11_f32_ap)
    bc_ps = cx.ps_misc[0:P, 0:1]
    nc.tensor.matmul(bc_ps, cx.ones_row[0:1, 0:P].bitcast(F32), s_r[:],
                     start=True, stop=True)
    col = p["fix"].tile([P, 1], F32, tag=cx.tag(tagbase))
    nc.vector.tensor_copy(col[:], bc_ps)
    return col


def emit_sigma(cx, wt_sb, K, M, name):
    """1/sigma (spectral norm, 1 power iter) for WT layout [K, M-per-ktile]:
    k-tile j lives at wt_sb[:, j*M:(j+1)*M].  Returns [128,1] f32 column."""
    nc, p = cx.nc, cx.p
    kt = (K + P - 1) // P
    mh = (M + P - 1) // P
    kp = min(P, K)

    vtmp = p["fix"].tile([kp, kt], F32, tag=cx.tag("sg_vt"))
    for j in range(kt):
        pt = min(P, K - P * j)
        nc.vector.tensor_reduce(vtmp[0:pt, j:j + 1], wt_sb[0:pt, j * M:(j + 1) * M],
                                axis=AX.X, op=ALU.add)
    vcol = p["fix"].tile([kp, kt], F32, tag=cx.tag("sg_vc"))
    nc.vector.tensor_scalar(vcol[:], vtmp[:], 1.0 / float(np.sqrt(M)), None, ALU.mult)

    s_ps = cx.ps_misc[0:1, 0:1]
    for j in range(kt):
        pt = min(P, K - P * j)
        nc.tensor.matmul(s_ps, vcol[0:pt, j:j + 1], vcol[0:pt, j:j + 1],
                         start=(j == 0), stop=(j == kt - 1))
    s_sb = p["fix"].tile([1, 1], F32, tag=cx.tag("sg_s"))
    nc.vector.tensor_copy(s_sb[:], s_ps)
    a_sb = p["fix"].tile([1, 1], F32, tag=cx.tag("sg_a"))
    nc.scalar.sqrt(a_sb[:], s_sb[:])
    d_sb = p["fix"].tile([1, 1], F32, tag=cx.tag("sg_d"))
    nc.vector.tensor_scalar(d_sb[:], a_sb[:], 1e-8, None, ALU.add)
    r_sb = p["fix"].tile([1, 1], F32, tag=cx.tag("sg_r"))
    nc.vector.reciprocal(r_sb[:], d_sb[:])
    rcol = emit_bcast11(cx, r_sb[:], "sg_rc")

    vhat = p["fix"].tile([kp, kt], F32, tag=cx.tag("sg_vh"))
    nc.vector.tensor_scalar(vhat[:], vcol[:], rcol[0:kp, :], None, ALU.mult)

    wv_ps = cx.ps_misc[0:P, 0:mh]
    for h in range(mh):
        mw = min(P, M - P * h)
        for j in range(kt):
            pt = min(P, K - P * j)
            nc.tensor.matmul(
                cx.ps_misc[0:mw, h:h + 1],
                wt_sb[0:pt, j * M + h * P: j * M + h * P + mw].bitcast(F32),
                vhat[0:pt, j:j + 1],
                start=(h == 0 and j == 0), stop=(h == mh - 1 and j == kt - 1))
    wv_sb = p["fix"].tile([P, mh], F32, tag=cx.tag("sg_wv"))
    for h in range(mh):
        mw = min(P, M - P * h)
        nc.vector.tensor_copy(wv_sb[0:mw, h:h + 1], cx.ps_misc[0:mw, h:h + 1])

    s2_ps = cx.ps_misc[0:1, 0:1]
    for h in range(mh):
        mw = min(P, M - P * h)
        nc.tensor.matmul(s2_ps, wv_sb[0:mw, h:h + 1], wv_sb[0:mw, h:h + 1],
                         start=(h == 0), stop=(h == mh - 1))
    s2_sb = p["fix"].tile([1, 1], F32, tag=cx.tag("sg_s2"))
    nc.vector.tensor_copy(s2_sb[:], s2_ps)
    a2 = p["fix"].tile([1, 1], F32, tag=cx.tag("sg_a2"))
    nc.scalar.sqrt(a2[:], s2_sb[:])
    d2 = p["fix"].tile([1, 1], F32, tag=cx.tag("sg_d2"))
    nc.vector.tensor_scalar(d2[:], a2[:], 1e-8, None, ALU.add)
    r2 = p["fix"].tile([1, 1], F32, tag=cx.tag("sg_r2"))
    nc.vector.reciprocal(r2[:], s2_sb[:])
    inv11 = p["fix"].tile([1, 1], F32, tag=cx.tag("sg_i"))
    nc.vector.tensor_mul(inv11[:], d2[:], r2[:])
    return emit_bcast11(cx, inv11[:], f"invs_{name}")


def emit_load_weight(cx, dram, K, M, name):
    """DMA WT [K, M] -> SBUF k-tiles along free dim; compute 1/sigma."""
    nc, p = cx.nc, cx.p
    kt = (K + P - 1) // P
    if K > 256:
        wt_sb = p["wbig"].tile([P, kt * M], F32R, tag="wbig")
    else:
        wt_sb = p["fix"].tile([min(P, K), kt * M], F32R, tag=cx.tag(f"w_{name}"))
    for j in range(kt):
        pt = min(P, K - P * j)
        nc.sync.dma_start(out=wt_sb[0:pt, j * M:(j + 1) * M],
                          in_=dram[j * P: j * P + pt, :])
    invs = emit_sigma(cx, wt_sb, K, M, name)
    return wt_sb, invs


def emit_load_cb(cx, dram):
    """Codebook [256,1024] -> (cbA, cbB, e2neg) with e2neg = -|e_k|^2/2."""
    nc, p = cx.nc, cx.p
    cbA = p["cbA"].tile([P, 1024], F32R, tag="cbA")
    cbB = p["cbB"].tile([P, 1024], F32R, tag="cbB")
    nc.sync.dma_start(out=cbA[:], in_=dram[0:128, :])
    nc.sync.dma_start(out=cbB[:], in_=dram[128:256, :])
    e2neg = p["e2"].tile([1, 1024], F32R, tag="e2")
    for kh in range(2):
        ksl = slice(512 * kh, 512 * kh + 512)
        sq = p["tsb"].tile([P, 1024], F32R, tag="tsb")
        nc.vector.tensor_tensor(sq[:, 0:512], cbA[:, ksl].bitcast(F32),
                                cbA[:, ksl].bitcast(F32), ALU.mult)
        nc.vector.tensor_tensor(sq[:, 512:1024], cbB[:, ksl].bitcast(F32),
                                cbB[:, ksl].bitcast(F32), ALU.mult)
        e2_ps = cx.ps_misc[0:1, 0:512]
        nc.tensor.matmul(e2_ps, cx.ones_col, sq[:, 0:512], start=True, stop=False)
        nc.tensor.matmul(e2_ps, cx.ones_col, sq[:, 512:1024], start=False, stop=True)
        nc.scalar.activation(e2neg[0:1, ksl], e2_ps, ACTF.Copy,
                             bias=0.0, scale=-0.5)
    return cbA, cbB, e2neg


def emit_vq(cx, h_cmp, cbA, cbB, e2neg, n_pix, pix_per_img, c_scale, qd_ps,
            c_acc, qd_close=False, vq_mode="full"):
    """VQ pass over compact activations h_cmp = 2 x [128, n_pix] (f32r).

    T[pix, code] = x.e - |e|^2/2;  min_dist = |x|^2 - 2*max_code(T).
    Accumulates c*sum(h^2) - 2c*sum(Tmax) per image into qd_ps[0, img].
    If c_acc is not None, adds code histogram counts into it ([1,1024] f32).
    """
    nc, p = cx.nc, cx.p
    ntile = n_pix // P
    n_img = n_pix // pix_per_img
    img_per_tile = max(1, P // pix_per_img)
    tile_per_img = max(1, pix_per_img // P)
    cpos = cx.cpos0 if c_scale == C0 else cx.cpos1
    cneg = cx.cneg0 if c_scale == C0 else cx.cneg1

    for i in range(n_img):
        if vq_mode == "T":
            break
        for coh in range(2):
            hsl = h_cmp[coh][:, i * pix_per_img:(i + 1) * pix_per_img].bitcast(F32)
            dump = p["dump"].tile([P, pix_per_img], F32, tag="dump")
            sqa = p["cols"].tile([P, 1], F32, tag="cols")
            nc.vector.tensor_tensor(dump[:], hsl, hsl, ALU.mult)
            nc.vector.tensor_reduce(sqa[:], dump[:], axis=AX.X, op=ALU.add)
            if vq_mode == "ttr":
                nc.sync.dma_start(out=cx.dbg_T[:, 2 * i + coh: 2 * i + coh + 1],
                                  in_=sqa[:])
                continue
            nc.tensor.matmul(qd_ps[0:1, i:i + 1], sqa[:], cpos.bitcast(F32),
                             start=cx.qd_first,
                             stop=(vq_mode == "sq" and i == n_img - 1 and coh == 1))
            cx.qd_first = False

    if vq_mode in ("sq", "ttr"):
        return
    for t in range(ntile):
        pix0 = t * P
        T_sb = p["tsb"].tile([P, 1024], F32, tag="tsb")
        for kh in range(2):
            ksl = slice(512 * kh, 512 * kh + 512)
            T_ps = p["vq"].tile([P, 512], F32, tag="vq")
            nc.tensor.matmul(T_ps[:], h_cmp[0][:, pix0:pix0 + P], cbA[:, ksl],
                             start=True, stop=False)
            nc.tensor.matmul(T_ps[:], h_cmp[1][:, pix0:pix0 + P], cbB[:, ksl],
                             start=False, stop=False)
            nc.tensor.matmul(T_ps[:], cx.ones_row[0:1, 0:P], e2neg[0:1, ksl],
                             start=False, stop=True)
            nc.scalar.copy(T_sb[:, ksl], T_ps[:])

        tmx = p["cols"].tile([P, 1], F32, tag="cols")
        nc.vector.tensor_reduce(tmx[:], T_sb[:], axis=AX.X, op=ALU.max)
        if vq_mode == "T":
            nc.sync.dma_start(out=cx.dbg_T[:, t:t + 1], in_=tmx[:])
            continue
        last = qd_close and t == ntile - 1
        if img_per_tile <= 1:
            i = t // tile_per_img
            nc.tensor.matmul(qd_ps[0:1, i:i + 1], tmx[:], cneg.bitcast(F32),
                             start=False, stop=last)
        else:
            assert img_per_tile == 2
            i0 = t * 2
            nc.tensor.matmul(qd_ps[0:1, i0:i0 + 2], tmx[:],
                             cx.ind2.bitcast(F32), start=False, stop=last)

        if c_acc is not None:
            eq = p["tsb"].tile([P, 1024], F32R, tag="tsb")
            nc.vector.tensor_scalar(eq[:], T_sb[:], tmx[:], None, ALU.is_equal)
            for kh in range(2):
                cnt_ps = p["sc"].tile([1, 512], F32, tag="sc")
                nc.tensor.matmul(cnt_ps[:], cx.ones_col,
                                 eq[:, 512 * kh:512 * kh + 512],
                                 start=True, stop=True)
                nc.vector.tensor_tensor(c_acc[0:1, 512 * kh:512 * kh + 512],
                                        c_acc[0:1, 512 * kh:512 * kh + 512],
                                        cnt_ps[:], ALU.add)


def emit_pool2(cx, src_ap, rows, cols):
    """2x2 sum-pool of SBUF tensor [128, rows*cols] -> dump tile [128, rows*cols/4]."""
    nc, p = cx.nc, cx.p
    hc = cols // 2
    hr = rows // 2
    t1 = p["dump"].tile([P, rows * hc], F32, tag="dump")
    v = src_ap.rearrange("p (r c two) -> p r c two", r=rows, c=hc, two=2)
    nc.vector.tensor_tensor(t1[:], v[:, :, :, 0], v[:, :, :, 1], ALU.add)
    t1v = t1[:].rearrange("p (r two c) -> p r two c", r=hr, two=2, c=hc)
    t2 = p["dump"].tile([P, hr * hc], F32, tag="dump")
    nc.vector.tensor_tensor(t2[:], t1v[:, :, 0, :], t1v[:, :, 1, :], ALU.add)
    return t2


def emit_pool_fuse(cx, y_ps_ap, sc_sb_ap, h_out_ap, rows, cols, scale_col,
                   bsum_col):
    """h_out = poolsum2x2(y_ps * scale_col) + bsum_col + sc_sb."""
    nc, p = cx.nc, cx.p
    y_sb = p["dump"].tile([P, rows * cols], F32, tag="dump")
    nc.scalar.activation(y_sb[:], y_ps_ap, ACTF.Identity, bias=0.0,
                         scale=scale_col[:])
    t2 = emit_pool2(cx, y_sb[:], rows, cols)
    nc.vector.scalar_tensor_tensor(h_out_ap, t2[:], bsum_col, sc_sb_ap,
                                   op0=ALU.add, op1=ALU.add)


DBG_WIDTH = 4096 + 1024 + 1024 + 1024 + 16


def build_module(dev=0, parts=4):
    nc = bacc.Bacc("TRN2", target_bir_lowering=False, debug=False)
    D = {}

    def din(name, shape, dt=F32R):
        D[name] = nc.dram_tensor(name, shape, dt, kind="ExternalInput")

    din("x8", [BPC, 3, 32, 32])
    din("w1t_b0", [27, 256])
    for nm in ("w2t_b0", "w1t_b1", "w2t_b1", "w1t_b2", "w2t_b2", "w1t_b3",
               "w2t_b3"):
        din(nm, [2304, 256])
    din("wsct_b0", [3, 256])
    din("wsct_b1", [256, 256])
    din("biases", [256, 10], F32)
    for i in range(4):
        din(f"cb{i}", [256, 1024])
    din("linT", [256, 1], F32)
    din("lin_b", [1, 1], F32)
    din("embT", [256, 100], F32)
    din("emb_nat", [100, 256], F32)
    din("y8", [1, BPC], F32)
    din("iota100", [100, 1], F32)
    din("cc", [P, 8])
    din("ones_row_d", [1, P])
    din("zpad", [P, 1024])

    o_out = nc.dram_tensor("o_out", [1, BPC], F32, kind="ExternalOutput")
    o_quant = nc.dram_tensor("o_quant", [1, BPC], F32, kind="ExternalOutput")
    o_counts = nc.dram_tensor("o_counts", [1, 1024], F32, kind="ExternalOutput")
    o_dbg = None
    if dev:
        o_dbg = nc.dram_tensor("o_dbg", [P, DBG_WIDTH], F32, kind="ExternalOutput")

    with tile.TileContext(nc) as tc:
        with (
            tc.tile_pool(name="fix", bufs=1) as fix,
            tc.tile_pool(name="wbig", bufs=2) as wbig,
            tc.tile_pool(name="cbA", bufs=2) as cbA_p,
            tc.tile_pool(name="cbB", bufs=2) as cbB_p,
            tc.tile_pool(name="e2", bufs=2) as e2_p,
            tc.tile_pool(name="tsb", bufs=2) as tsb,
            tc.tile_pool(name="dump", bufs=6) as dump,
            tc.tile_pool(name="cols", bufs=8) as cols,
            tc.tile_pool(name="conv", bufs=3, space="PSUM") as conv_ps,
            tc.tile_pool(name="scps", bufs=1, space="PSUM") as sc_ps,
            tc.tile_pool(name="vq", bufs=2, space="PSUM") as vq_psp,
            tc.tile_pool(name="qd", bufs=1, space="PSUM") as qd_p,
            tc.tile_pool(name="misc", bufs=1, space="PSUM") as misc,
        ):
            pools = dict(fix=fix, wbig=wbig, cbA=cbA_p, cbB=cbB_p, e2=e2_p,
                         tsb=tsb, dump=dump, cols=cols, conv=conv_ps, sc=sc_ps,
                         vq=vq_psp, qd=qd_p, misc=misc)
            cx = Ctx(nc, tc, pools, D)
            emit_all(cx, D, o_out, o_quant, o_counts, o_dbg, parts)

    nc.compile()
    return nc


def emit_all(cx, D, o_out, o_quant, o_counts, o_dbg, parts=4):
    nc, p = cx.nc, cx.p

    # ---- consts / biases
    cc = p["fix"].tile([P, 8], F32R, tag="cc")
    nc.sync.dma_start(out=cc[:], in_=D["cc"][:, :])
    cx.ones_col = cc[:, 0:1]
    cx.cpos0 = cc[:, 1:2]
    cx.cneg0 = cc[:, 2:3]
    cx.cpos1 = cc[:, 3:4]
    cx.cneg1 = cc[:, 4:5]
    cx.ind2 = cc[:, 6:8]
    ones_row = p["fix"].tile([1, P], F32R, tag="ones_row")
    nc.sync.dma_start(out=ones_row[:], in_=D["ones_row_d"][:, :])
    cx.ones_row = ones_row

    bias = p["fix"].tile([P, 20], F32, tag="bias")
    nc.sync.dma_start(out=bias[:, 0:10], in_=D["biases"][0:128, :])
    nc.sync.dma_start(out=bias[:, 10:20], in_=D["biases"][128:256, :])

    def bcol(idx, coh):
        return bias[:, 10 * coh + idx: 10 * coh + idx + 1]

    bsum = p["fix"].tile([P, 4], F32, tag="bsum")
    nc.vector.tensor_tensor(bsum[:, 0:1], bcol(2, 0), bcol(1, 0), ALU.add)
    nc.vector.tensor_tensor(bsum[:, 1:2], bcol(2, 1), bcol(1, 1), ALU.add)
    nc.vector.tensor_tensor(bsum[:, 2:3], bcol(5, 0), bcol(4, 0), ALU.add)
    nc.vector.tensor_tensor(bsum[:, 3:4], bcol(5, 1), bcol(4, 1), ALU.add)

    qd_full = p["qd"].tile([1, 512], F32, tag="qd")
    qd_ps = qd_full[0:1, 0:BPC]
    cx.ps_misc = p["misc"].tile([P, 512], F32, tag="misc")
    c_acc = p["fix"].tile([1, 1024], F32, tag="c_acc")
    nc.vector.memset(c_acc[:], 0.0)

    # ---- block0 weights
    w1, is1 = emit_load_weight(cx, D["w1t_b0"], 27, 256, "b0w1")
    w2, is2 = emit_load_weight(cx, D["w2t_b0"], 2304, 256, "b0w2")
    wsc, issc = emit_load_weight(cx, D["wsct_b0"], 3, 256, "b0wsc")
    cb0A, cb0B, e2n0 = emit_load_cb(cx, D["cb0"])

    def scale4(col, nm):
        out = p["fix"].tile([P, 1], F32, tag=cx.tag(nm))
        nc.vector.tensor_scalar(out[:], col[:], 0.25, None, ALU.mult)
        return out

    is2_4 = scale4(is2, "is2_4")
    issc_4 = scale4(issc, "issc_4")

    # im2col buffers, zeroed once (tap-valid regions identical per image)
    X0 = []
    for b in range(2):
        t = p["fix"].tile([27, 1024], F32R, tag=f"X0_{b}")
        nc.sync.dma_start(out=t[:], in_=D["zpad"][0:27, 0:1024])
        X0.append(t)

    def zero_borders(t, side, nimg):
        if nimg == 1:
            v = t[:].rearrange("p (r c) -> p r c", r=side, c=side)
            nc.sync.dma_start(out=v[:, 0, :], in_=D["zpad"][:, 0:side])
            nc.sync.dma_start(out=v[:, side - 1, :], in_=D["zpad"][:, 0:side])
            nc.sync.dma_start(out=v[:, 1:side - 1, 0], in_=D["zpad"][:, 0:side - 2])
            nc.sync.dma_start(out=v[:, 1:side - 1, side - 1],
                              in_=D["zpad"][:, 0:side - 2])
        else:
            v = t[:].rearrange("p (i r c) -> p i r c", i=nimg, r=side, c=side)
            for i in range(nimg):
                nc.sync.dma_start(out=v[:, i, 0, :], in_=D["zpad"][:, 0:side])
                nc.sync.dma_start(out=v[:, i, side - 1, :], in_=D["zpad"][:, 0:side])
                nc.sync.dma_start(out=v[:, i, 1:side - 1, 0],
                                  in_=D["zpad"][:, 0:side - 2])
                nc.sync.dma_start(out=v[:, i, 1:side - 1, side - 1],
                                  in_=D["zpad"][:, 0:side - 2])

    def padded_plane(tagbase, side, nimg):
        pair = []
        for coh in range(2):
            t = p["fix"].tile([P, nimg * side * side], F32R,
                              tag=f"{tagbase}_{coh}")
            zero_borders(t, side, nimg)
            pair.append(t)
        return pair

    y1p = []
    for b in range(2):
        pair = []
        for coh in range(2):
            t = p["fix"].tile([P, 34 * 34], F32R, tag=f"y1p_{b}_{coh}")
            zero_borders(t, 34, 1)
            pair.append(t)
        y1p.append(pair)

    rh0 = padded_plane("rh0", 18, BPC)
    h0c = [p["fix"].tile([P, BPC * 256], F32R, tag=f"h0c_{c}", name=f"h0c_{c}") for c in range(2)]

    # ---------------- block 0 ----------------
    if parts < 0.2:
        if o_dbg is not None:
            nc.sync.dma_start(out=o_dbg[:, 0:1], in_=is1[:])
            nc.sync.dma_start(out=o_dbg[:, 1:2], in_=is2[:])
            nc.sync.dma_start(out=o_dbg[:, 2:3], in_=issc[:])
            nc.sync.dma_start(out=o_dbg[0:1, 16:1040], in_=e2n0[:].bitcast(F32))
        return
    dma_eng = [nc.sync, nc.sync, nc.sync]
    for i in range(BPC):
        xb = X0[i % 2]
        dstv = xb[:].rearrange("p (r c) -> p r c", r=32, c=32)
        for pos, t9 in enumerate(TAP_ORDER):
            dy, dx = t9 // 3, t9 % 3
            hlo, hhi = max(0, 1 - dy), min(32, 33 - dy)
            wlo, whi = max(0, 1 - dx), min(32, 33 - dx)
            dma_eng[pos % 3].dma_start(
                out=dstv[3 * pos:3 * pos + 3, hlo:hhi, wlo:whi],
                in_=D["x8"][i, :, hlo + dy - 1:hhi + dy - 1,
                            wlo + dx - 1:whi + dx - 1])
        yb = y1p[i % 2]
        if parts < 0.4:
            if o_dbg is not None:
                nc.sync.dma_start(out=o_dbg[0:27, 1040 + i * 128:1040 + i * 128 + 128],
                                  in_=xb[:, 0:128].bitcast(F32))
            continue
        for s in range(2):
            for coh in range(2):
                ps = p["conv"].tile([P, 512], F32, tag="conv")
                nc.tensor.matmul(ps[:], w1[0:27, 128 * coh:128 * coh + 128],
                                 xb[:, 512 * s:512 * s + 512], start=True, stop=True)
                ov = yb[coh][:].rearrange("p (r c) -> p r c", r=34, c=34)
                nc.scalar.activation(ov[:, 1 + 16 * s:17 + 16 * s, 1:33], ps[:],
                                     ACTF.Relu, bias=bcol(0, coh), scale=is1[:])
        if parts < 0.6:
            if o_dbg is not None:
                nc.sync.dma_start(out=o_dbg[:, 1040 + i * 128:1040 + i * 128 + 128],
                                  in_=yb[0][:, 0:128].bitcast(F32))
            continue
        # pooled x (sum form) from im2col center tap (rows 12..14 = full image)
        xt1 = p["dump"].tile([3, 512], F32, tag="dump")
        xcv = xb[0:3, :].rearrange("p (r c two) -> p r c two", r=32, c=16, two=2)
        nc.vector.tensor_tensor(xt1[:], xcv[:, :, :, 0], xcv[:, :, :, 1], ALU.add)
        xp_i = p["dump"].tile([3, 256], F32R, tag="dump")
        xt1v = xt1[:].rearrange("p (r two c) -> p r two c", r=16, two=2, c=16)
        nc.vector.tensor_tensor(xp_i[:], xt1v[:, :, 0, :], xt1v[:, :, 1, :], ALU.add)
        sc_psT = p["sc"].tile([P, 512], F32, tag="sc")
        for coh in range(2):
            nc.tensor.matmul(sc_psT[:, 256 * coh:256 * coh + 256],
                             wsc[0:3, 128 * coh:128 * coh + 128],
                             xp_i[:], start=(coh == 0), stop=(coh == 1))
        for s in range(2):
            c2 = []
            for coh in range(2):
                ps = p["conv"].tile([P, 512], F32, tag="conv")
                first = True
                for t9 in range(9):
                    dy, dx = t9 // 3, t9 % 3
                    for cih in range(2):
                        src = yb[cih][:].rearrange("p (r c) -> p r c", r=34, c=34)
                        rhs = src[:, dy + 16 * s: dy + 16 * s + 16, dx:dx + 32]
                        nc.tensor.matmul(
                            ps[:],
                            w2[:, (2 * t9 + cih) * 256 + 128 * coh:
                                  (2 * t9 + cih) * 256 + 128 * coh + 128],
                            rhs, start=first, stop=(t9 == 8 and cih == 1))
                        first = False
                c2.append(ps)
            for coh in range(2):
                sc_sb = p["dump"].tile([P, 128], F32, tag="dump")
                scv = sc_psT[:, 256 * coh:256 * coh + 256].rearrange(
                    "p (r c) -> p r c", r=16, c=16)
                nc.scalar.activation(
                    sc_sb[:].rearrange("p (r c) -> p r c", r=8, c=16),
                    scv[:, 8 * s:8 * s + 8, :],
                    ACTF.Identity, bias=0.0, scale=issc_4[:])
                emit_pool_fuse(
                    cx, c2[coh][:], sc_sb[:],
                    h0c[coh][:, i * 256 + 128 * s: i * 256 + 128 * s + 128],
                    16, 32, is2_4, bsum[:, coh:coh + 1])
        for coh in range(2):
            ov = rh0[coh][:].rearrange("p (i r c) -> p i r c", i=BPC, r=18, c=18)
            nc.scalar.activation(
                ov[:, i, 1:17, 1:17],
                h0c[coh][:, i * 256:(i + 1) * 256].bitcast(F32).rearrange(
                    "p (r c) -> p r c", r=16, c=16),
                ACTF.Relu, bias=0.0, scale=1.0)

    if parts in (0.8, 0.85, 0.9):
        pass
    elif parts < 1:
        if o_dbg is not None and parts >= 0.6:
            for t, w, off in ((h0c[0], 2048, 0), (h0c[1], 2048, 2048)):
                nc.sync.dma_start(out=o_dbg[:, off:off + w], in_=t[:].bitcast(F32))
        return
    if parts in (0.8, 0.85, 0.9):
        cx.dbg_T = o_dbg
        emit_vq(cx, h0c, cb0A, cb0B, e2n0, BPC * 256, 256, C0, qd_ps, None,
                qd_close=False,
                vq_mode={0.8: "sq", 0.85: "ttr", 0.9: "T"}[parts])
        if parts == 0.8:
            q_sb = p["fix"].tile([1, BPC], F32, tag="q_sb")
            nc.scalar.copy(q_sb[:], qd_ps)
            nc.sync.dma_start(out=o_quant[:, :], in_=q_sb[:])
        return
    emit_vq(cx, h0c, cb0A, cb0B, e2n0, BPC * 256, 256, C0, qd_ps, None,
            qd_close=(parts == 1))
    if parts >= 2:
        emit_rest(cx, D, o_out, o_quant, o_counts, o_dbg, parts,
                  bcol, bsum, qd_ps, c_acc, h0c, rh0, padded_plane, scale4,
                  zero_borders)
    else:
        q_sb = p["fix"].tile([1, BPC], F32, tag="q_sb")
        nc.scalar.copy(q_sb[:], qd_ps)
        nc.sync.dma_start(out=o_quant[:, :], in_=q_sb[:])
        if o_dbg is not None:
            for t, w, off in ((h0c[0], 2048, 0), (h0c[1], 2048, 2048)):
                nc.sync.dma_start(out=o_dbg[:, off:off + w], in_=t[:].bitcast(F32))


def emit_rest(cx, D, o_out, o_quant, o_counts, o_dbg, parts,
              bcol, bsum, qd_ps, c_acc, h0c, rh0, padded_plane, scale4,
              zero_borders):
    nc, p = cx.nc, cx.p
    # ---------------- block 1 ----------------
    w1b1, is1b1 = emit_load_weight(cx, D["w1t_b1"], 2304, 256, "b1w1")
    w2b1, is2b1 = emit_load_weight(cx, D["w2t_b1"], 2304, 256, "b1w2")
    wscb1, isscb1 = emit_load_weight(cx, D["wsct_b1"], 256, 256, "b1wsc")
    cb1A, cb1B, e2n1 = emit_load_cb(cx, D["cb1"])
    is2b1_4 = scale4(is2b1, "is2b1_4")
    isscb1_4 = scale4(isscb1, "isscb1_4")

    rt1 = []
    for b in range(2):
        pair = []
        for coh in range(2):
            t = p["fix"].tile([P, 18 * 18], F32R, tag=f"rt1_{b}_{coh}")
            zero_borders(t, 18, 1)
            pair.append(t)
        rt1.append(pair)

    rh1 = padded_plane("rh1", 10, BPC)
    h1c = [p["fix"].tile([P, BPC * 64], F32R, tag=f"h1c_{c}", name=f"h1c_{c}") for c in range(2)]

    for i in range(BPC):
        rb = rt1[i % 2]
        for coh in range(2):
            psf = p["conv"].tile([P, 512], F32, tag="conv")
            ps = psf[:, 0:256]
            first = True
            for t9 in range(9):
                dy, dx = t9 // 3, t9 % 3
                for cih in range(2):
                    src = rh0[cih][:].rearrange("p (i r c) -> p i r c",
                                                i=BPC, r=18, c=18)
                    rhs = src[:, i, dy:dy + 16, dx:dx + 16]
                    nc.tensor.matmul(
                        ps[:],
                        w1b1[:, (2 * t9 + cih) * 256 + 128 * coh:
                                (2 * t9 + cih) * 256 + 128 * coh + 128],
                        rhs, start=first, stop=(t9 == 8 and cih == 1))
                    first = False
            ov = rb[coh][:].rearrange("p (r c) -> p r c", r=18, c=18)
            nc.scalar.activation(
                ov[:, 1:17, 1:17],
                ps[:].rearrange("p (r c) -> p r c", r=16, c=16),
                ACTF.Relu, bias=bcol(3, coh), scale=is1b1[:])
        sc_psT = p["sc"].tile([P, 512], F32, tag="sc")
        for coh in range(2):
            for cih in range(2):
                nc.tensor.matmul(
                    sc_psT[:, 256 * coh:256 * coh + 256],
                    wscb1[:, cih * 256 + 128 * coh: cih * 256 + 128 * coh + 128],
                    h0c[cih][:, i * 256:(i + 1) * 256],
                    start=(coh == 0 and cih == 0), stop=(coh == 1 and cih == 1))
        for coh in range(2):
            psf = p["conv"].tile([P, 512], F32, tag="conv")
            ps = psf[:, 0:256]
            first = True
            for t9 in range(9):
                dy, dx = t9 // 3, t9 % 3
                for cih in range(2):
                    src = rb[cih][:].rearrange("p (r c) -> p r c", r=18, c=18)
                    rhs = src[:, dy:dy + 16, dx:dx + 16]
                    nc.tensor.matmul(
                        ps[:],
                        w2b1[:, (2 * t9 + cih) * 256 + 128 * coh:
                                (2 * t9 + cih) * 256 + 128 * coh + 128],
                        rhs, start=first, stop=(t9 == 8 and cih == 1))
                    first = False
            sc_e = p["dump"].tile([P, 256], F32, tag="dump")
            nc.scalar.activation(sc_e[:], sc_psT[:, 256 * coh:256 * coh + 256],
                                 ACTF.Identity, bias=0.0, scale=isscb1_4[:])
            sc_t2 = emit_pool2(cx, sc_e[:], 16, 16)
            emit_pool_fuse(cx, ps[:], sc_t2[:],
                           h1c[coh][:, i * 64:(i + 1) * 64], 16, 16, is2b1_4,
                           bsum[:, 2 + coh:3 + coh])
        for coh in range(2):
            ov = rh1[coh][:].rearrange("p (i r c) -> p i r c", i=BPC, r=10, c=10)
            nc.scalar.activation(
                ov[:, i, 1:9, 1:9],
                h1c[coh][:, i * 64:(i + 1) * 64].bitcast(F32).rearrange(
                    "p (r c) -> p r c", r=8, c=8),
                ACTF.Relu, bias=0.0, scale=1.0)

    emit_vq(cx, h1c, cb1A, cb1B, e2n1, BPC * 64, 64, C1, qd_ps, None,
            qd_close=(parts == 2))
    if parts < 3:
        q_sb = p["fix"].tile([1, BPC], F32, tag="q_sb")
        nc.scalar.copy(q_sb[:], qd_ps)
        nc.sync.dma_start(out=o_quant[:, :], in_=q_sb[:])
        if o_dbg is not None:
            dl = ((h0c[0], 2048, 0), (h0c[1], 2048, 2048),
                  (h1c[0], 512, 4096), (h1c[1], 512, 4608))
            for t, w, off in dl:
                nc.sync.dma_start(out=o_dbg[:, off:off + w], in_=t[:].bitcast(F32))
        return

    # ---------------- blocks 2 & 3 ----------------
    def emit_block23(rh_in, h_in, w1d, w2d, cbd, b1i, b2i, nm, c_acc_arg):
        w1x, is1x = emit_load_weight(cx, w1d, 2304, 256, nm + "w1")
        w2x, is2x = emit_load_weight(cx, w2d, 2304, 256, nm + "w2")
        cbXA, cbXB, e2nX = emit_load_cb(cx, cbd)
        rt = padded_plane("rt_" + nm, 10, BPC)
        h_out = [p["fix"].tile([P, BPC * 64], F32R, tag=f"h_{nm}_{c}",
                               name=f"h_{nm}_{c}") for c in range(2)]
        for coh in range(2):
            ps = p["conv"].tile([P, 512], F32, tag="conv")
            first = True
            for t9 in range(9):
                dy, dx = t9 // 3, t9 % 3
                for cih in range(2):
                    src = rh_in[cih][:].rearrange("p (i r c) -> p i r c",
                                                  i=BPC, r=10, c=10)
                    rhs = src[:, :, dy:dy + 8, dx:dx + 8]
                    nc.tensor.matmul(
                        ps[:],
                        w1x[:, (2 * t9 + cih) * 256 + 128 * coh:
                                (2 * t9 + cih) * 256 + 128 * coh + 128],
                        rhs, start=first, stop=(t9 == 8 and cih == 1))
                    first = False
            ov = rt[coh][:].rearrange("p (i r c) -> p i r c", i=BPC, r=10, c=10)
            nc.scalar.activation(
                ov[:, :, 1:9, 1:9],
                ps[:].rearrange("p (i r c) -> p i r c", i=BPC, r=8, c=8),
                ACTF.Relu, bias=bcol(b1i, coh), scale=is1x[:])
        for coh in range(2):
            ps = p["conv"].tile([P, 512], F32, tag="conv")
            first = True
            for t9 in range(9):
                dy, dx = t9 // 3, t9 % 3
                for cih in range(2):
                    src = rt[cih][:].rearrange("p (i r c) -> p i r c",
                                               i=BPC, r=10, c=10)
                    rhs = src[:, :, dy:dy + 8, dx:dx + 8]
                    nc.tensor.matmul(
                        ps[:],
                        w2x[:, (2 * t9 + cih) * 256 + 128 * coh:
                                (2 * t9 + cih) * 256 + 128 * coh + 128],
                        rhs, start=first, stop=(t9 == 8 and cih == 1))
                    first = False
            tmp = p["dump"].tile([P, 512], F32, tag="dump")
            nc.scalar.activation(tmp[:], ps[:], ACTF.Identity,
                                 bias=bcol(b2i, coh), scale=is2x[:])
            nc.vector.tensor_tensor(h_out[coh][:], tmp[:],
                                    h_in[coh][:].bitcast(F32), ALU.add)
        emit_vq(cx, h_out, cbXA, cbXB, e2nX, BPC * 64, 64, C1, qd_ps, c_acc_arg,
                qd_close=(c_acc_arg is not None))
        return h_out

    h2c = emit_block23(rh1, h1c, D["w1t_b2"], D["w2t_b2"], D["cb2"], 6, 7,
                       "b2", None)
    rh2 = padded_plane("rh2", 10, BPC)
    for coh in range(2):
        ov = rh2[coh][:].rearrange("p (i r c) -> p i r c", i=BPC, r=10, c=10)
        nc.scalar.activation(
            ov[:, :, 1:9, 1:9],
            h2c[coh][:].bitcast(F32).rearrange("p (i r c) -> p i r c",
                                               i=BPC, r=8, c=8),
            ACTF.Relu, bias=0.0, scale=1.0)
    h3c = emit_block23(rh2, h2c, D["w1t_b3"], D["w2t_b3"], D["cb3"], 8, 9,
                       "b3", c_acc)
    if parts < 4:
        q_sb = p["fix"].tile([1, BPC], F32, tag="q_sb")
        nc.scalar.copy(q_sb[:], qd_ps)
        nc.sync.dma_start(out=o_quant[:, :], in_=q_sb[:])
        nc.sync.dma_start(out=o_counts[:, :], in_=c_acc[:])
        if o_dbg is not None:
            dl = ((h0c[0], 2048, 0), (h0c[1], 2048, 2048),
                  (h1c[0], 512, 4096), (h1c[1], 512, 4608),
                  (h2c[0], 512, 5120), (h2c[1], 512, 5632),
                  (h3c[0], 512, 6144), (h3c[1], 512, 6656))
            for t, w, off in dl:
                nc.sync.dma_start(out=o_dbg[:, off:off + w], in_=t[:].bitcast(F32))
        return

    # ---------------- head ----------------
    hf = [p["fix"].tile([P, BPC], F32, tag=f"hf_{c}", name=f"hf_{c}") for c in range(2)]
    for coh in range(2):
        for i in range(BPC):
            dmp = p["dump"].tile([P, 64], F32, tag="dump")
            nc.scalar.activation(dmp[:],
                                 h3c[coh][:, i * 64:(i + 1) * 64].bitcast(F32),
                                 ACTF.Relu, bias=0.0, scale=1.0,
                                 accum_out=hf[coh][:, i:i + 1])

    linT = p["fix"].tile([P, 2], F32, tag="linT")
    nc.sync.dma_start(out=linT[:, 0:1], in_=D["linT"][0:128, :])
    nc.sync.dma_start(out=linT[:, 1:2], in_=D["linT"][128:256, :])
    isl = emit_sigma(cx, linT, 256, 1, "lin")
    embT = p["fix"].tile([P, 200], F32, tag="embT")
    nc.sync.dma_start(out=embT[:, 0:100], in_=D["embT"][0:128, :])
    nc.sync.dma_start(out=embT[:, 100:200], in_=D["embT"][128:256, :])
    ise = emit_sigma(cx, embT, 256, 100, "emb")

    y_sb = p["fix"].tile([1, BPC], F32, tag="y_sb")
    nc.sync.dma_start(out=y_sb[:], in_=D["y8"][:, :])
    iot = p["fix"].tile([100, 1], F32, tag="iot")
    nc.sync.dma_start(out=iot[:], in_=D["iota100"][:, :])
    ybc_ps = cx.ps_misc[0:100, 0:BPC]
    nc.tensor.matmul(ybc_ps, cx.ones_row[0:1, 0:100].bitcast(F32), y_sb[:],
                     start=True, stop=True)
    oh = p["fix"].tile([100, BPC], F32, tag="oh")
    nc.vector.tensor_scalar(oh[:], ybc_ps, iot[:], None, ALU.is_equal)
    emb_sb = p["fix"].tile([100, 256], F32, tag="emb_sb")
    nc.sync.dma_start(out=emb_sb[:], in_=D["emb_nat"][:, :])

    Fq = []
    for coh in range(2):
        es_ps = cx.ps_misc[0:P, 0:BPC]
        nc.tensor.matmul(es_ps, emb_sb[:, 128 * coh:128 * coh + 128], oh[:],
                         start=True, stop=True)
        linsc = p["fix"].tile([P, 1], F32, tag=cx.tag("linsc"))
        nc.vector.tensor_scalar(linsc[:], linT[:, coh:coh + 1],
                                isl[:], None, ALU.mult)
        A_sb = p["fix"].tile([P, BPC], F32, tag=cx.tag("A_sb"))
        nc.scalar.activation(A_sb[:], es_ps, ACTF.Identity,
                             bias=linsc[:], scale=ise[:])
        f = p["fix"].tile([P, BPC], F32, tag=cx.tag("Fq"))
        nc.vector.tensor_tensor(f[:], hf[coh][:], A_sb[:], ALU.mult)
        Fq.append(f)
    out_ps = cx.ps_misc[0:1, 0:BPC]
    for coh in range(2):
        nc.tensor.matmul(out_ps, cx.ones_col.bitcast(F32), Fq[coh][:],
                         start=(coh == 0), stop=(coh == 1))
    lb = p["fix"].tile([1, 1], F32, tag="lb")
    nc.sync.dma_start(out=lb[:], in_=D["lin_b"][:, :])
    o_sb = p["fix"].tile([1, BPC], F32, tag="o_sb")
    nc.scalar.activation(o_sb[:], out_ps, ACTF.Identity, bias=lb[:], scale=1.0)
    nc.sync.dma_start(out=o_out[:, :], in_=o_sb[:])

    q_sb = p["fix"].tile([1, BPC], F32, tag="q_sb")
    nc.scalar.copy(q_sb[:], qd_ps[:])
    nc.sync.dma_start(out=o_quant[:, :], in_=q_sb[:])
    nc.sync.dma_start(out=o_counts[:, :], in_=c_acc[:])

    if o_dbg is not None:
        off = 0
        for t, w in ((h0c[0], 2048), (h0c[1], 2048), (h1c[0], 512), (h1c[1], 512),
                     (h2c[0], 512), (h2c[1], 512), (h3c[0], 512), (h3c[1], 512),
                     (hf[0], 8), (hf[1], 8)):
            nc.sync.dma_start(out=o_dbg[:, off:off + w], in_=t[:].bitcast(F32))
            off += w


# ----------------------------------------------------------------- host side

_NC_CACHE = {}


def _get_nc(dev=0, parts=4):
    key = (dev, parts)
    if key not in _NC_CACHE:
        _NC_CACHE[key] = build_module(dev, parts)
    return _NC_CACHE[key]


def _wt3x3(w):
    co, ci = w.shape[0], w.shape[1]
    return _r32r(np.ascontiguousarray(
        np.asarray(w).reshape(co, ci, 3, 3).transpose(2, 3, 1, 0).reshape(
            9 * ci, co)))


def prep_core_inputs(inputs, core):
    d = {}
    s = slice(core * BPC, (core + 1) * BPC)
    d["x8"] = _r32r(inputs["x"][s])
    w1b0 = np.asarray(inputs["b0_w1"]).reshape(256, 3, 3, 3).transpose(2, 3, 1, 0)
    w1b0 = w1b0.reshape(9, 3, 256)[TAP_ORDER].reshape(27, 256)
    d["w1t_b0"] = _r32r(np.ascontiguousarray(w1b0))
    d["w2t_b0"] = _wt3x3(inputs["b0_w2"])
    d["w1t_b1"] = _wt3x3(inputs["b1_w1"])
    d["w2t_b1"] = _wt3x3(inputs["b1_w2"])
    d["w1t_b2"] = _wt3x3(inputs["b2_w1"])
    d["w2t_b2"] = _wt3x3(inputs["b2_w2"])
    d["w1t_b3"] = _wt3x3(inputs["b3_w1"])
    d["w2t_b3"] = _wt3x3(inputs["b3_w2"])
    d["wsct_b0"] = _r32r(np.asarray(inputs["b0_wsc"]).reshape(256, 3).T)
    d["wsct_b1"] = _r32r(np.asarray(inputs["b1_wsc"]).reshape(256, 256).T)
    biases = np.stack([
        inputs["b0_b1"], inputs["b0_b2"], inputs["b0_bsc"],
        inputs["b1_b1"], inputs["b1_b2"], inputs["b1_bsc"],
        inputs["b2_b1"], inputs["b2_b2"],
        inputs["b3_b1"], inputs["b3_b2"]], axis=1).astype(np.float32)
    d["biases"] = np.ascontiguousarray(biases)
    for i in range(4):
        d[f"cb{i}"] = _r32r(inputs[f"cb{i}"])
    d["linT"] = _r32r(np.asarray(inputs["lin_w"]).reshape(1, 256).T)
    d["lin_b"] = np.asarray(inputs["lin_b"], np.float32).reshape(1, 1)
    d["embT"] = _r32r(np.asarray(inputs["emb_w"]).T)
    d["emb_nat"] = _r32r(inputs["emb_w"])
    d["y8"] = _r32r(np.asarray(inputs["y"][s], np.float32).reshape(1, BPC))
    d["iota100"] = np.arange(100, dtype=np.float32).reshape(100, 1)
    cc = np.zeros((P, 8), np.float32)
    cc[:, 0] = 1.0
    cc[:, 1] = C0
    cc[:, 2] = -2.0 * C0
    cc[:, 3] = C1
    cc[:, 4] = -2.0 * C1
    cc[0:64, 6] = -2.0 * C1
    cc[64:128, 7] = -2.0 * C1
    d["cc"] = _r32r(cc)
    d["ones_row_d"] = np.ones((1, P), np.float32)
    d["zpad"] = np.zeros((P, 1024), np.float32)
    return d


def run_cores(inputs, dev=0, **kw):
    nc = _get_nc(dev)
    in_maps = [prep_core_inputs(inputs, c) for c in range(NCORES)]
    return run_bass_kernel_spmd(nc, in_maps, core_ids=list(range(NCORES)), **kw)


def assemble(results):
    outs = np.concatenate([r["o_out"][0] for r in results]).reshape(64, 1)
    quant = np.concatenate([r["o_quant"][0] for r in results]).reshape(64, 1)
    counts = np.sum([r["o_counts"][0] for r in results], axis=0).astype(np.float32)
    probs = counts / np.float32(64 * 8 * 8)
    ppl = np.exp(-np.sum(probs * np.log(probs + np.float32(1e-10)),
                         dtype=np.float32)).astype(np.float32)
    return (outs.astype(np.float32), quant.astype(np.float32), ppl)


def kernel(**inputs):
    inputs = {k: np.asarray(v) for k, v in inputs.items()}
    res = run_cores(inputs)
    return assemble(res.results)


# revision 21
# speedup vs baseline: 1.2300x; 1.2300x over previous
"""Trainium2 Bass kernel for nn_Discriminator_61332132987171 (vq_codebook).

Data-parallel over batch: 8 images per NeuronCore across 8 cores.
All matmuls in float32r (fp32 with 11-bit mantissa, full PE rate at N>=256).

Per-core pipeline:
  block0: im2col conv 3->256 (K=27), conv 256->256 (9-tap accumulation over a
          zero-padded SBUF plane), avgpool2, 1x1-conv shortcut on pooled x
  VQ0..VQ3: T[pix,code] = x.e - |e|^2/2 via matmuls (codes on free dim),
          max-reduce over codes, per-image sums into one PSUM accumulator
  block1: preact block at 16x16 with downsample + 1x1 shortcut
  block2/3: preact blocks at 8x8, image-batched matmuls (3D moving APs)
  head: hf = sum relu(h3) via ACT accum_out; out = hf . (lin/sl + emb[y]/se) + b
  spectral-norm 1/sigma for every weight computed on device and folded into
  the PSUM-eviction activation scale.

Host side only shards/transposes/rounds inputs and reduces per-core histogram
counts into the final perplexity scalar.
"""
import sys

for _p in ("/opt/trn_rl_repo", "/opt/pypackages"):
    if _p not in sys.path:
        sys.path.append(_p)

import numpy as np
import concourse.bass as bass  # noqa: F401
import concourse.mybir as mybir
import concourse.tile as tile
from concourse import bacc
from concourse.bass_utils import run_bass_kernel_spmd

F32 = mybir.dt.float32
F32R = mybir.dt.float32r
AX = mybir.AxisListType
ALU = mybir.AluOpType
ACTF = mybir.ActivationFunctionType

P = 128
NCORES = 8
BPC = 8  # images per core

C0 = 0.5 / (256.0 * 16 * 16)   # quant-loss scale, block0
C1 = 0.5 / (256.0 * 8 * 8)     # blocks 1-3
TAP_ORDER = [4, 0, 1, 2, 3, 5, 6, 7, 8]  # center tap first (partitions 0..2)


def _r32r(x):
    """Round fp32 -> fp32r (11-bit mantissa, RTNE) on host."""
    u = np.ascontiguousarray(x, dtype=np.float32).view(np.uint32)
    u2 = u + 0x7FF + ((u >> 12) & 1)
    return (u2 & 0xFFFFF000).astype(np.uint32).view(np.float32)


class Ctx:
    def __init__(self, nc, tc, pools, inp):
        self.nc = nc
        self.tc = tc
        self.p = pools
        self.inp = inp
        self.uid = 0
        self.qd_first = True

    def tag(self, base):
        self.uid += 1
        return f"{base}{self.uid}"


def emit_bcast11(cx, src11_f32_ap, tagbase):
    """[1,1] f32 AP -> [128,1] f32 SBUF column (K=1 ones matmul broadcast)."""
    nc, p = cx.nc, cx.p
    s_r = p["fix"].tile([1, 1], F32, tag=cx.tag("bc_r"))
    nc.vector.tensor_copy(s_r[:], src11_f32_ap)
    bc_ps = cx.ps_misc[0:P, 0:1]
    nc.tensor.matmul(bc_ps, cx.ones_row[0:1, 0:P].bitcast(F32), s_r[:],
                     start=True, stop=True)
    col = p["fix"].tile([P, 1], F32, tag=cx.tag(tagbase))
    nc.vector.tensor_copy(col[:], bc_ps)
    return col


def emit_sigma(cx, wt_sb, K, M, name):
    """1/sigma (spectral norm, 1 power iter) for WT layout [K, M-per-ktile]:
    k-tile j lives at wt_sb[:, j*M:(j+1)*M].  Returns [128,1] f32 column."""
    nc, p = cx.nc, cx.p
    kt = (K + P - 1) // P
    mh = (M + P - 1) // P
    kp = min(P, K)

    vtmp = p["fix"].tile([kp, kt], F32, tag=cx.tag("sg_vt"))
    for j in range(kt):
        pt = min(P, K - P * j)
        nc.vector.tensor_reduce(vtmp[0:pt, j:j + 1], wt_sb[0:pt, j * M:(j + 1) * M],
                                axis=AX.X, op=ALU.add)
    vcol = p["fix"].tile([kp, kt], F32, tag=cx.tag("sg_vc"))
    nc.vector.tensor_scalar(vcol[:], vtmp[:], 1.0 / float(np.sqrt(M)), None, ALU.mult)

    s_ps = cx.ps_misc[0:1, 0:1]
    for j in range(kt):
        pt = min(P, K - P * j)
        nc.tensor.matmul(s_ps, vcol[0:pt, j:j + 1], vcol[0:pt, j:j + 1],
                         start=(j == 0), stop=(j == kt - 1))
    s_sb = p["fix"].tile([1, 1], F32, tag=cx.tag("sg_s"))
    nc.vector.tensor_copy(s_sb[:], s_ps)
    a_sb = p["fix"].tile([1, 1], F32, tag=cx.tag("sg_a"))
    nc.scalar.sqrt(a_sb[:], s_sb[:])
    d_sb = p["fix"].tile([1, 1], F32, tag=cx.tag("sg_d"))
    nc.vector.tensor_scalar(d_sb[:], a_sb[:], 1e-8, None, ALU.add)
    r_sb = p["fix"].tile([1, 1], F32, tag=cx.tag("sg_r"))
    nc.vector.reciprocal(r_sb[:], d_sb[:])
    rcol = emit_bcast11(cx, r_sb[:], "sg_rc")

    vhat = p["fix"].tile([kp, kt], F32, tag=cx.tag("sg_vh"))
    nc.vector.tensor_scalar(vhat[:], vcol[:], rcol[0:kp, :], None, ALU.mult)

    wv_ps = cx.ps_misc[0:P, 0:mh]
    for h in range(mh):
        mw = min(P, M - P * h)
        for j in range(kt):
            pt = min(P, K - P * j)
            nc.tensor.matmul(
                cx.ps_misc[0:mw, h:h + 1],
                wt_sb[0:pt, j * M + h * P: j * M + h * P + mw].bitcast(F32),
                vhat[0:pt, j:j + 1],
                start=(h == 0 and j == 0), stop=(h == mh - 1 and j == kt - 1))
    wv_sb = p["fix"].tile([P, mh], F32, tag=cx.tag("sg_wv"))
    for h in range(mh):
        mw = min(P, M - P * h)
        nc.vector.tensor_copy(wv_sb[0:mw, h:h + 1], cx.ps_misc[0:mw, h:h + 1])

    s2_ps = cx.ps_misc[0:1, 0:1]
    for h in range(mh):
        mw = min(P, M - P * h)
        nc.tensor.matmul(s2_ps, wv_sb[0:mw, h:h + 1], wv_sb[0:mw, h:h + 1],
                         start=(h == 0), stop=(h == mh - 1))
    s2_sb = p["fix"].tile([1, 1], F32, tag=cx.tag("sg_s2"))
    nc.vector.tensor_copy(s2_sb[:], s2_ps)
    a2 = p["fix"].tile([1, 1], F32, tag=cx.tag("sg_a2"))
    nc.scalar.sqrt(a2[:], s2_sb[:])
    d2 = p["fix"].tile([1, 1], F32, tag=cx.tag("sg_d2"))
    nc.vector.tensor_scalar(d2[:], a2[:], 1e-8, None, ALU.add)
    r2 = p["fix"].tile([1, 1], F32, tag=cx.tag("sg_r2"))
    nc.vector.reciprocal(r2[:], s2_sb[:])
    inv11 = p["fix"].tile([1, 1], F32, tag=cx.tag("sg_i"))
    nc.vector.tensor_mul(inv11[:], d2[:], r2[:])
    return emit_bcast11(cx, inv11[:], f"invs_{name}")


def emit_load_weight(cx, dram, K, M, name):
    """DMA WT [K, M] -> SBUF k-tiles along free dim; compute 1/sigma."""
    nc, p = cx.nc, cx.p
    kt = (K + P - 1) // P
    if K > 256:
        wt_sb = p["wbig"].tile([P, kt * M], F32R, tag="wbig")
    else:
        wt_sb = p["fix"].tile([min(P, K), kt * M], F32R, tag=cx.tag(f"w_{name}"))
    for j in range(kt):
        pt = min(P, K - P * j)
        nc.sync.dma_start(out=wt_sb[0:pt, j * M:(j + 1) * M],
                          in_=dram[j * P: j * P + pt, :])
    invs = emit_sigma(cx, wt_sb, K, M, name)
    return wt_sb, invs


def emit_load_cb(cx, dram):
    """Codebook [256,1024] -> (cbA, cbB, e2neg) with e2neg = -|e_k|^2/2."""
    nc, p = cx.nc, cx.p
    cbA = p["cbA"].tile([P, 1024], F32R, tag="cbA")
    cbB = p["cbB"].tile([P, 1024], F32R, tag="cbB")
    nc.sync.dma_start(out=cbA[:], in_=dram[0:128, :])
    nc.sync.dma_start(out=cbB[:], in_=dram[128:256, :])
    e2neg = p["e2"].tile([1, 1024], F32R, tag="e2")
    for kh in range(2):
        ksl = slice(512 * kh, 512 * kh + 512)
        sq = p["tsb"].tile([P, 1024], F32R, tag="tsb")
        nc.vector.tensor_tensor(sq[:, 0:512], cbA[:, ksl].bitcast(F32),
                                cbA[:, ksl].bitcast(F32), ALU.mult)
        nc.vector.tensor_tensor(sq[:, 512:1024], cbB[:, ksl].bitcast(F32),
                                cbB[:, ksl].bitcast(F32), ALU.mult)
        e2_ps = cx.ps_misc[0:1, 0:512]
        nc.tensor.matmul(e2_ps, cx.ones_col, sq[:, 0:512], start=True, stop=False)
        nc.tensor.matmul(e2_ps, cx.ones_col, sq[:, 512:1024], start=False, stop=True)
        nc.scalar.activation(e2neg[0:1, ksl], e2_ps, ACTF.Copy,
                             bias=0.0, scale=-0.5)
    return cbA, cbB, e2neg


def emit_vq(cx, h_cmp, cbA, cbB, e2neg, n_pix, pix_per_img, c_scale, qd_ps,
            c_acc, qd_close=False, vq_mode="full"):
    """VQ pass over compact activations h_cmp = 2 x [128, n_pix] (f32r).

    T[pix, code] = x.e - |e|^2/2;  min_dist = |x|^2 - 2*max_code(T).
    Accumulates c*sum(h^2) - 2c*sum(Tmax) per image into qd_ps[0, img].
    If c_acc is not None, adds code histogram counts into it ([1,1024] f32).
    """
    nc, p = cx.nc, cx.p
    ntile = n_pix // P
    n_img = n_pix // pix_per_img
    img_per_tile = max(1, P // pix_per_img)
    tile_per_img = max(1, pix_per_img // P)
    cpos = cx.cpos0 if c_scale == C0 else cx.cpos1
    cneg = cx.cneg0 if c_scale == C0 else cx.cneg1

    for i in range(n_img):
        if vq_mode == "T":
            break
        for coh in range(2):
            hsl = h_cmp[coh][:, i * pix_per_img:(i + 1) * pix_per_img].bitcast(F32)
            dump = p["dump"].tile([P, pix_per_img], F32, tag="dump")
            sqa = p["cols"].tile([P, 1], F32, tag="cols")
            nc.vector.tensor_tensor(dump[:], hsl, hsl, ALU.mult)
            nc.vector.tensor_reduce(sqa[:], dump[:], axis=AX.X, op=ALU.add)
            if vq_mode == "ttr":
                nc.sync.dma_start(out=cx.dbg_T[:, 2 * i + coh: 2 * i + coh + 1],
                                  in_=sqa[:])
                continue
            nc.tensor.matmul(qd_ps[0:1, i:i + 1], sqa[:], cpos.bitcast(F32),
                             start=cx.qd_first,
                             stop=(vq_mode == "sq" and i == n_img - 1 and coh == 1))
            cx.qd_first = False

    if vq_mode in ("sq", "ttr"):
        return
    for t in range(ntile):
        pix0 = t * P
        T_sb = p["tsb"].tile([P, 1024], F32, tag="tsb")
        for kh in range(2):
            ksl = slice(512 * kh, 512 * kh + 512)
            T_ps = p["vq"].tile([P, 512], F32, tag="vq")
            nc.tensor.matmul(T_ps[:], h_cmp[0][:, pix0:pix0 + P], cbA[:, ksl],
                             start=True, stop=False)
            nc.tensor.matmul(T_ps[:], h_cmp[1][:, pix0:pix0 + P], cbB[:, ksl],
                             start=False, stop=False)
            nc.tensor.matmul(T_ps[:], cx.ones_row[0:1, 0:P], e2neg[0:1, ksl],
                             start=False, stop=True)
            nc.scalar.copy(T_sb[:, ksl], T_ps[:])

        tmx = p["cols"].tile([P, 1], F32, tag="cols")
        nc.vector.tensor_reduce(tmx[:], T_sb[:], axis=AX.X, op=ALU.max)
        if vq_mode == "T":
            nc.sync.dma_start(out=cx.dbg_T[:, t:t + 1], in_=tmx[:])
            continue
        last = qd_close and t == ntile - 1
        if img_per_tile <= 1:
            i = t // tile_per_img
            nc.tensor.matmul(qd_ps[0:1, i:i + 1], tmx[:], cneg.bitcast(F32),
                             start=False, stop=last)
        else:
            assert img_per_tile == 2
            i0 = t * 2
            nc.tensor.matmul(qd_ps[0:1, i0:i0 + 2], tmx[:],
                             cx.ind2.bitcast(F32), start=False, stop=last)

        if c_acc is not None:
            eq = p["tsb"].tile([P, 1024], F32R, tag="tsb")
            nc.vector.tensor_scalar(eq[:], T_sb[:], tmx[:], None, ALU.is_equal)
            for kh in range(2):
                cnt_ps = p["sc"].tile([1, 512], F32, tag="sc")
                nc.tensor.matmul(cnt_ps[:], cx.ones_col,
                                 eq[:, 512 * kh:512 * kh + 512],
                                 start=True, stop=True)
                nc.vector.tensor_tensor(c_acc[0:1, 512 * kh:512 * kh + 512],
                                        c_acc[0:1, 512 * kh:512 * kh + 512],
                                        cnt_ps[:], ALU.add)


def emit_pool2(cx, src_ap, rows, cols):
    """2x2 sum-pool of SBUF tensor [128, rows*cols] -> dump tile [128, rows*cols/4]."""
    nc, p = cx.nc, cx.p
    hc = cols // 2
    hr = rows // 2
    t1 = p["dump"].tile([P, rows * hc], F32, tag="dump")
    v = src_ap.rearrange("p (r c two) -> p r c two", r=rows, c=hc, two=2)
    nc.vector.tensor_tensor(t1[:], v[:, :, :, 0], v[:, :, :, 1], ALU.add)
    t1v = t1[:].rearrange("p (r two c) -> p r two c", r=hr, two=2, c=hc)
    t2 = p["dump"].tile([P, hr * hc], F32, tag="dump")
    nc.vector.tensor_tensor(t2[:], t1v[:, :, 0, :], t1v[:, :, 1, :], ALU.add)
    return t2


def emit_pool_fuse(cx, y_ps_ap, sc_sb_ap, h_out_ap, rows, cols, scale_col,
                   bsum_col):
    """h_out = poolsum2x2(y_ps * scale_col) + bsum_col + sc_sb."""
    nc, p = cx.nc, cx.p
    y_sb = p["dump"].tile([P, rows * cols], F32, tag="dump")
    nc.scalar.activation(y_sb[:], y_ps_ap, ACTF.Identity, bias=0.0,
                         scale=scale_col[:])
    t2 = emit_pool2(cx, y_sb[:], rows, cols)
    nc.vector.scalar_tensor_tensor(h_out_ap, t2[:], bsum_col, sc_sb_ap,
                                   op0=ALU.add, op1=ALU.add)


DBG_WIDTH = 4096 + 1024 + 1024 + 1024 + 16


def build_module(dev=0, parts=4):
    nc = bacc.Bacc("TRN2", target_bir_lowering=False, debug=False)
    D = {}

    def din(name, shape, dt=F32R):
        D[name] = nc.dram_tensor(name, shape, dt, kind="ExternalInput")

    din("x8", [BPC, 3, 32, 32])
    din("w1t_b0", [27, 256])
    for nm in ("w2t_b0", "w1t_b1", "w2t_b1", "w1t_b2", "w2t_b2", "w1t_b3",
               "w2t_b3"):
        din(nm, [2304, 256])
    din("wsct_b0", [3, 256])
    din("wsct_b1", [256, 256])
    din("biases", [256, 10], F32)
    for i in range(4):
        din(f"cb{i}", [256, 1024])
    din("linT", [256, 1], F32)
    din("lin_b", [1, 1], F32)
    din("embT", [256, 100], F32)
    din("emb_nat", [100, 256], F32)
    din("y8", [1, BPC], F32)
    din("iota100", [100, 1], F32)
    din("cc", [P, 8])
    din("ones_row_d", [1, P])
    din("zpad", [P, 2592])

    o_out = nc.dram_tensor("o_out", [1, BPC], F32, kind="ExternalOutput")
    o_quant = nc.dram_tensor("o_quant", [1, BPC], F32, kind="ExternalOutput")
    o_counts = nc.dram_tensor("o_counts", [1, 1024], F32, kind="ExternalOutput")
    o_dbg = None
    if dev:
        o_dbg = nc.dram_tensor("o_dbg", [P, DBG_WIDTH], F32, kind="ExternalOutput")

    with tile.TileContext(nc) as tc:
        with (
            tc.tile_pool(name="fix", bufs=1) as fix,
            tc.tile_pool(name="wbig", bufs=2) as wbig,
            tc.tile_pool(name="cbA", bufs=2) as cbA_p,
            tc.tile_pool(name="cbB", bufs=2) as cbB_p,
            tc.tile_pool(name="e2", bufs=2) as e2_p,
            tc.tile_pool(name="tsb", bufs=2) as tsb,
            tc.tile_pool(name="dump", bufs=6) as dump,
            tc.tile_pool(name="cols", bufs=8) as cols,
            tc.tile_pool(name="conv", bufs=3, space="PSUM") as conv_ps,
            tc.tile_pool(name="scps", bufs=1, space="PSUM") as sc_ps,
            tc.tile_pool(name="vq", bufs=2, space="PSUM") as vq_psp,
            tc.tile_pool(name="qd", bufs=1, space="PSUM") as qd_p,
            tc.tile_pool(name="misc", bufs=1, space="PSUM") as misc,
        ):
            pools = dict(fix=fix, wbig=wbig, cbA=cbA_p, cbB=cbB_p, e2=e2_p,
                         tsb=tsb, dump=dump, cols=cols, conv=conv_ps, sc=sc_ps,
                         vq=vq_psp, qd=qd_p, misc=misc)
            cx = Ctx(nc, tc, pools, D)
            emit_all(cx, D, o_out, o_quant, o_counts, o_dbg, parts)

    nc.compile()
    return nc


def emit_all(cx, D, o_out, o_quant, o_counts, o_dbg, parts=4):
    nc, p = cx.nc, cx.p

    # ---- consts / biases
    cc = p["fix"].tile([P, 8], F32R, tag="cc")
    nc.sync.dma_start(out=cc[:], in_=D["cc"][:, :])
    cx.ones_col = cc[:, 0:1]
    cx.cpos0 = cc[:, 1:2]
    cx.cneg0 = cc[:, 2:3]
    cx.cpos1 = cc[:, 3:4]
    cx.cneg1 = cc[:, 4:5]
    cx.ind2 = cc[:, 6:8]
    ones_row = p["fix"].tile([1, P], F32R, tag="ones_row")
    nc.sync.dma_start(out=ones_row[:], in_=D["ones_row_d"][:, :])
    cx.ones_row = ones_row

    bias = p["fix"].tile([P, 20], F32, tag="bias")
    nc.sync.dma_start(out=bias[:, 0:10], in_=D["biases"][0:128, :])
    nc.sync.dma_start(out=bias[:, 10:20], in_=D["biases"][128:256, :])

    def bcol(idx, coh):
        return bias[:, 10 * coh + idx: 10 * coh + idx + 1]

    bsum = p["fix"].tile([P, 4], F32, tag="bsum")
    nc.vector.tensor_tensor(bsum[:, 0:1], bcol(2, 0), bcol(1, 0), ALU.add)
    nc.vector.tensor_tensor(bsum[:, 1:2], bcol(2, 1), bcol(1, 1), ALU.add)
    nc.vector.tensor_tensor(bsum[:, 2:3], bcol(5, 0), bcol(4, 0), ALU.add)
    nc.vector.tensor_tensor(bsum[:, 3:4], bcol(5, 1), bcol(4, 1), ALU.add)

    qd_full = p["qd"].tile([1, 512], F32, tag="qd")
    qd_ps = qd_full[0:1, 0:BPC]
    cx.ps_misc = p["misc"].tile([P, 512], F32, tag="misc")
    c_acc = p["fix"].tile([1, 1024], F32, tag="c_acc")
    nc.vector.memset(c_acc[:], 0.0)

    # ---- block0 weights
    w1, is1 = emit_load_weight(cx, D["w1t_b0"], 27, 256, "b0w1")
    w2, is2 = emit_load_weight(cx, D["w2t_b0"], 2304, 256, "b0w2")
    wsc, issc = emit_load_weight(cx, D["wsct_b0"], 3, 256, "b0wsc")
    cb0A, cb0B, e2n0 = emit_load_cb(cx, D["cb0"])

    def scale4(col, nm):
        out = p["fix"].tile([P, 1], F32, tag=cx.tag(nm))
        nc.vector.tensor_scalar(out[:], col[:], 0.25, None, ALU.mult)
        return out

    is2_4 = scale4(is2, "is2_4")
    issc_4 = scale4(issc, "issc_4")

    # im2col buffers, zeroed once (tap-valid regions identical per image)
    X0 = []
    for b in range(2):
        t = p["fix"].tile([27, 1024], F32R, tag=f"X0_{b}")
        nc.sync.dma_start(out=t[:], in_=D["zpad"][0:27, 0:1024])
        X0.append(t)

    def zero_borders(t, side, nimg):
        n = nimg * side * side
        nc.sync.dma_start(out=t[:, 0:n], in_=D["zpad"][:, 0:n])

    def padded_plane(tagbase, side, nimg):
        pair = []
        for coh in range(2):
            t = p["fix"].tile([P, nimg * side * side], F32R,
                              tag=f"{tagbase}_{coh}")
            zero_borders(t, side, nimg)
            pair.append(t)
        return pair

    y1p = []
    for b in range(2):
        pair = []
        for coh in range(2):
            t = p["fix"].tile([P, 34 * 34], F32R, tag=f"y1p_{b}_{coh}")
            zero_borders(t, 34, 1)
            pair.append(t)
        y1p.append(pair)

    rh0 = padded_plane("rh0", 18, BPC)
    h0c = [p["fix"].tile([P, BPC * 256], F32R, tag=f"h0c_{c}", name=f"h0c_{c}") for c in range(2)]

    # ---------------- block 0 ----------------
    if parts < 0.2:
        if o_dbg is not None:
            nc.sync.dma_start(out=o_dbg[:, 0:1], in_=is1[:])
            nc.sync.dma_start(out=o_dbg[:, 1:2], in_=is2[:])
            nc.sync.dma_start(out=o_dbg[:, 2:3], in_=issc[:])
            nc.sync.dma_start(out=o_dbg[0:1, 16:1040], in_=e2n0[:].bitcast(F32))
        return
    dma_eng = [nc.sync, nc.sync, nc.sync]
    for i in range(BPC):
        xb = X0[i % 2]
        dstv = xb[:].rearrange("p (r c) -> p r c", r=32, c=32)
        for pos, t9 in enumerate(TAP_ORDER):
            dy, dx = t9 // 3, t9 % 3
            hlo, hhi = max(0, 1 - dy), min(32, 33 - dy)
            wlo, whi = max(0, 1 - dx), min(32, 33 - dx)
            dma_eng[pos % 3].dma_start(
                out=dstv[3 * pos:3 * pos + 3, hlo:hhi, wlo:whi],
                in_=D["x8"][i, :, hlo + dy - 1:hhi + dy - 1,
                            wlo + dx - 1:whi + dx - 1])
        yb = y1p[i % 2]
        if parts < 0.4:
            if o_dbg is not None:
                nc.sync.dma_start(out=o_dbg[0:27, 1040 + i * 128:1040 + i * 128 + 128],
                                  in_=xb[:, 0:128].bitcast(F32))
            continue
        for s in range(2):
            for coh in range(2):
                ps = p["conv"].tile([P, 512], F32, tag="conv")
                nc.tensor.matmul(ps[:], w1[0:27, 128 * coh:128 * coh + 128],
                                 xb[:, 512 * s:512 * s + 512], start=True, stop=True)
                ov = yb[coh][:].rearrange("p (r c) -> p r c", r=34, c=34)
                nc.scalar.activation(ov[:, 1 + 16 * s:17 + 16 * s, 1:33], ps[:],
                                     ACTF.Relu, bias=bcol(0, coh), scale=is1[:])
        if parts < 0.6:
            if o_dbg is not None:
                nc.sync.dma_start(out=o_dbg[:, 1040 + i * 128:1040 + i * 128 + 128],
                                  in_=yb[0][:, 0:128].bitcast(F32))
            continue
        # pooled x (sum form) from im2col center tap (rows 12..14 = full image)
        xt1 = p["dump"].tile([3, 512], F32, tag="dump")
        xcv = xb[0:3, :].rearrange("p (r c two) -> p r c two", r=32, c=16, two=2)
        nc.vector.tensor_tensor(xt1[:], xcv[:, :, :, 0], xcv[:, :, :, 1], ALU.add)
        xp_i = p["dump"].tile([3, 256], F32R, tag="dump")
        xt1v = xt1[:].rearrange("p (r two c) -> p r two c", r=16, two=2, c=16)
        nc.vector.tensor_tensor(xp_i[:], xt1v[:, :, 0, :], xt1v[:, :, 1, :], ALU.add)
        sc_psT = p["sc"].tile([P, 512], F32, tag="sc")
        for coh in range(2):
            nc.tensor.matmul(sc_psT[:, 256 * coh:256 * coh + 256],
                             wsc[0:3, 128 * coh:128 * coh + 128],
                             xp_i[:], start=(coh == 0), stop=(coh == 1))
        for s in range(2):
            c2 = []
            for coh in range(2):
                ps = p["conv"].tile([P, 512], F32, tag="conv")
                first = True
                for t9 in range(9):
                    dy, dx = t9 // 3, t9 % 3
                    for cih in range(2):
                        src = yb[cih][:].rearrange("p (r c) -> p r c", r=34, c=34)
                        rhs = src[:, dy + 16 * s: dy + 16 * s + 16, dx:dx + 32]
                        nc.tensor.matmul(
                            ps[:],
                            w2[:, (2 * t9 + cih) * 256 + 128 * coh:
                                  (2 * t9 + cih) * 256 + 128 * coh + 128],
                            rhs, start=first, stop=(t9 == 8 and cih == 1))
                        first = False
                c2.append(ps)
            for coh in range(2):
                sc_sb = p["dump"].tile([P, 128], F32, tag="dump")
                scv = sc_psT[:, 256 * coh:256 * coh + 256].rearrange(
                    "p (r c) -> p r c", r=16, c=16)
                nc.scalar.activation(
                    sc_sb[:].rearrange("p (r c) -> p r c", r=8, c=16),
                    scv[:, 8 * s:8 * s + 8, :],
                    ACTF.Identity, bias=0.0, scale=issc_4[:])
                emit_pool_fuse(
                    cx, c2[coh][:], sc_sb[:],
                    h0c[coh][:, i * 256 + 128 * s: i * 256 + 128 * s + 128],
                    16, 32, is2_4, bsum[:, coh:coh + 1])
        for coh in range(2):
            ov = rh0[coh][:].rearrange("p (i r c) -> p i r c", i=BPC, r=18, c=18)
            nc.scalar.activation(
                ov[:, i, 1:17, 1:17],
                h0c[coh][:, i * 256:(i + 1) * 256].bitcast(F32).rearrange(
                    "p (r c) -> p r c", r=16, c=16),
                ACTF.Relu, bias=0.0, scale=1.0)

    if parts in (0.8, 0.85, 0.9):
        pass
    elif parts < 1:
        if o_dbg is not None and parts >= 0.6:
            for t, w, off in ((h0c[0], 2048, 0), (h0c[1], 2048, 2048)):
                nc.sync.dma_start(out=o_dbg[:, off:off + w], in_=t[:].bitcast(F32))
        return
    if parts in (0.8, 0.85, 0.9):
        cx.dbg_T = o_dbg
        emit_vq(cx, h0c, cb0A, cb0B, e2n0, BPC * 256, 256, C0, qd_ps, None,
                qd_close=False,
                vq_mode={0.8: "sq", 0.85: "ttr", 0.9: "T"}[parts])
        if parts == 0.8:
            q_sb = p["fix"].tile([1, BPC], F32, tag="q_sb")
            nc.scalar.copy(q_sb[:], qd_ps)
            nc.sync.dma_start(out=o_quant[:, :], in_=q_sb[:])
        return
    emit_vq(cx, h0c, cb0A, cb0B, e2n0, BPC * 256, 256, C0, qd_ps, None,
            qd_close=(parts == 1))
    if parts >= 2:
        emit_rest(cx, D, o_out, o_quant, o_counts, o_dbg, parts,
                  bcol, bsum, qd_ps, c_acc, h0c, rh0, padded_plane, scale4,
                  zero_borders)
    else:
        q_sb = p["fix"].tile([1, BPC], F32, tag="q_sb")
        nc.scalar.copy(q_sb[:], qd_ps)
        nc.sync.dma_start(out=o_quant[:, :], in_=q_sb[:])
        if o_dbg is not None:
            for t, w, off in ((h0c[0], 2048, 0), (h0c[1], 2048, 2048)):
                nc.sync.dma_start(out=o_dbg[:, off:off + w], in_=t[:].bitcast(F32))


def emit_rest(cx, D, o_out, o_quant, o_counts, o_dbg, parts,
              bcol, bsum, qd_ps, c_acc, h0c, rh0, padded_plane, scale4,
              zero_borders):
    nc, p = cx.nc, cx.p
    # ---------------- block 1 ----------------
    w1b1, is1b1 = emit_load_weight(cx, D["w1t_b1"], 2304, 256, "b1w1")
    w2b1, is2b1 = emit_load_weight(cx, D["w2t_b1"], 2304, 256, "b1w2")
    wscb1, isscb1 = emit_load_weight(cx, D["wsct_b1"], 256, 256, "b1wsc")
    cb1A, cb1B, e2n1 = emit_load_cb(cx, D["cb1"])
    is2b1_4 = scale4(is2b1, "is2b1_4")
    isscb1_4 = scale4(isscb1, "isscb1_4")

    rt1 = []
    for b in range(2):
        pair = []
        for coh in range(2):
            t = p["fix"].tile([P, 18 * 18], F32R, tag=f"rt1_{b}_{coh}")
            zero_borders(t, 18, 1)
            pair.append(t)
        rt1.append(pair)

    rh1 = padded_plane("rh1", 10, BPC)
    h1c = [p["fix"].tile([P, BPC * 64], F32R, tag=f"h1c_{c}", name=f"h1c_{c}") for c in range(2)]

    for i in range(BPC):
        rb = rt1[i % 2]
        for coh in range(2):
            psf = p["conv"].tile([P, 512], F32, tag="conv")
            ps = psf[:, 0:256]
            first = True
            for t9 in range(9):
                dy, dx = t9 // 3, t9 % 3
                for cih in range(2):
                    src = rh0[cih][:].rearrange("p (i r c) -> p i r c",
                                                i=BPC, r=18, c=18)
                    rhs = src[:, i, dy:dy + 16, dx:dx + 16]
                    nc.tensor.matmul(
                        ps[:],
                        w1b1[:, (2 * t9 + cih) * 256 + 128 * coh:
                                (2 * t9 + cih) * 256 + 128 * coh + 128],
                        rhs, start=first, stop=(t9 == 8 and cih == 1))
                    first = False
            ov = rb[coh][:].rearrange("p (r c) -> p r c", r=18, c=18)
            nc.scalar.activation(
                ov[:, 1:17, 1:17],
                ps[:].rearrange("p (r c) -> p r c", r=16, c=16),
                ACTF.Relu, bias=bcol(3, coh), scale=is1b1[:])
        sc_psT = p["sc"].tile([P, 512], F32, tag="sc")
        for coh in range(2):
            for cih in range(2):
                nc.tensor.matmul(
                    sc_psT[:, 256 * coh:256 * coh + 256],
                    wscb1[:, cih * 256 + 128 * coh: cih * 256 + 128 * coh + 128],
                    h0c[cih][:, i * 256:(i + 1) * 256],
                    start=(coh == 0 and cih == 0), stop=(coh == 1 and cih == 1))
        for coh in range(2):
            psf = p["conv"].tile([P, 512], F32, tag="conv")
            ps = psf[:, 0:256]
            first = True
            for t9 in range(9):
                dy, dx = t9 // 3, t9 % 3
                for cih in range(2):
                    src = rb[cih][:].rearrange("p (r c) -> p r c", r=18, c=18)
                    rhs = src[:, dy:dy + 16, dx:dx + 16]
                    nc.tensor.matmul(
                        ps[:],
                        w2b1[:, (2 * t9 + cih) * 256 + 128 * coh:
                                (2 * t9 + cih) * 256 + 128 * coh + 128],
                        rhs, start=first, stop=(t9 == 8 and cih == 1))
                    first = False
            sc_e = p["dump"].tile([P, 256], F32, tag="dump")
            nc.scalar.activation(sc_e[:], sc_psT[:, 256 * coh:256 * coh + 256],
                                 ACTF.Identity, bias=0.0, scale=isscb1_4[:])
            sc_t2 = emit_pool2(cx, sc_e[:], 16, 16)
            emit_pool_fuse(cx, ps[:], sc_t2[:],
                           h1c[coh][:, i * 64:(i + 1) * 64], 16, 16, is2b1_4,
                           bsum[:, 2 + coh:3 + coh])
        for coh in range(2):
            ov = rh1[coh][:].rearrange("p (i r c) -> p i r c", i=BPC, r=10, c=10)
            nc.scalar.activation(
                ov[:, i, 1:9, 1:9],
                h1c[coh][:, i * 64:(i + 1) * 64].bitcast(F32).rearrange(
                    "p (r c) -> p r c", r=8, c=8),
                ACTF.Relu, bias=0.0, scale=1.0)

    emit_vq(cx, h1c, cb1A, cb1B, e2n1, BPC * 64, 64, C1, qd_ps, None,
            qd_close=(parts == 2))
    if parts < 3:
        q_sb = p["fix"].tile([1, BPC], F32, tag="q_sb")
        nc.scalar.copy(q_sb[:], qd_ps)
        nc.sync.dma_start(out=o_quant[:, :], in_=q_sb[:])
        if o_dbg is not None:
            dl = ((h0c[0], 2048, 0), (h0c[1], 2048, 2048),
                  (h1c[0], 512, 4096), (h1c[1], 512, 4608))
            for t, w, off in dl:
                nc.sync.dma_start(out=o_dbg[:, off:off + w], in_=t[:].bitcast(F32))
        return

    # ---------------- blocks 2 & 3 ----------------
    def emit_block23(rh_in, h_in, w1d, w2d, cbd, b1i, b2i, nm, c_acc_arg):
        w1x, is1x = emit_load_weight(cx, w1d, 2304, 256, nm + "w1")
        w2x, is2x = emit_load_weight(cx, w2d, 2304, 256, nm + "w2")
        cbXA, cbXB, e2nX = emit_load_cb(cx, cbd)
        rt = padded_plane("rt_" + nm, 10, BPC)
        h_out = [p["fix"].tile([P, BPC * 64], F32R, tag=f"h_{nm}_{c}",
                               name=f"h_{nm}_{c}") for c in range(2)]
        for coh in range(2):
            ps = p["conv"].tile([P, 512], F32, tag="conv")
            first = True
            for t9 in range(9):
                dy, dx = t9 // 3, t9 % 3
                for cih in range(2):
                    src = rh_in[cih][:].rearrange("p (i r c) -> p i r c",
                                                  i=BPC, r=10, c=10)
                    rhs = src[:, :, dy:dy + 8, dx:dx + 8]
                    nc.tensor.matmul(
                        ps[:],
                        w1x[:, (2 * t9 + cih) * 256 + 128 * coh:
                                (2 * t9 + cih) * 256 + 128 * coh + 128],
                        rhs, start=first, stop=(t9 == 8 and cih == 1))
                    first = False
            ov = rt[coh][:].rearrange("p (i r c) -> p i r c", i=BPC, r=10, c=10)
            nc.scalar.activation(
                ov[:, :, 1:9, 1:9],
                ps[:].rearrange("p (i r c) -> p i r c", i=BPC, r=8, c=8),
                ACTF.Relu, bias=bcol(b1i, coh), scale=is1x[:])
        for coh in range(2):
            ps = p["conv"].tile([P, 512], F32, tag="conv")
            first = True
            for t9 in range(9):
                dy, dx = t9 // 3, t9 % 3
                for cih in range(2):
                    src = rt[cih][:].rearrange("p (i r c) -> p i r c",
                                               i=BPC, r=10, c=10)
                    rhs = src[:, :, dy:dy + 8, dx:dx + 8]
                    nc.tensor.matmul(
                        ps[:],
                        w2x[:, (2 * t9 + cih) * 256 + 128 * coh:
                                (2 * t9 + cih) * 256 + 128 * coh + 128],
                        rhs, start=first, stop=(t9 == 8 and cih == 1))
                    first = False
            tmp = p["dump"].tile([P, 512], F32, tag="dump")
            nc.scalar.activation(tmp[:], ps[:], ACTF.Identity,
                                 bias=bcol(b2i, coh), scale=is2x[:])
            nc.vector.tensor_tensor(h_out[coh][:], tmp[:],
                                    h_in[coh][:].bitcast(F32), ALU.add)
        emit_vq(cx, h_out, cbXA, cbXB, e2nX, BPC * 64, 64, C1, qd_ps, c_acc_arg,
                qd_close=(c_acc_arg is not None))
        return h_out

    h2c = emit_block23(rh1, h1c, D["w1t_b2"], D["w2t_b2"], D["cb2"], 6, 7,
                       "b2", None)
    rh2 = padded_plane("rh2", 10, BPC)
    for coh in range(2):
        ov = rh2[coh][:].rearrange("p (i r c) -> p i r c", i=BPC, r=10, c=10)
        nc.scalar.activation(
            ov[:, :, 1:9, 1:9],
            h2c[coh][:].bitcast(F32).rearrange("p (i r c) -> p i r c",
                                               i=BPC, r=8, c=8),
            ACTF.Relu, bias=0.0, scale=1.0)
    h3c = emit_block23(rh2, h2c, D["w1t_b3"], D["w2t_b3"], D["cb3"], 8, 9,
                       "b3", c_acc)
    if parts < 4:
        q_sb = p["fix"].tile([1, BPC], F32, tag="q_sb")
        nc.scalar.copy(q_sb[:], qd_ps)
        nc.sync.dma_start(out=o_quant[:, :], in_=q_sb[:])
        nc.sync.dma_start(out=o_counts[:, :], in_=c_acc[:])
        if o_dbg is not None:
            dl = ((h0c[0], 2048, 0), (h0c[1], 2048, 2048),
                  (h1c[0], 512, 4096), (h1c[1], 512, 4608),
                  (h2c[0], 512, 5120), (h2c[1], 512, 5632),
                  (h3c[0], 512, 6144), (h3c[1], 512, 6656))
            for t, w, off in dl:
                nc.sync.dma_start(out=o_dbg[:, off:off + w], in_=t[:].bitcast(F32))
        return

    # ---------------- head ----------------
    hf = [p["fix"].tile([P, BPC], F32, tag=f"hf_{c}", name=f"hf_{c}") for c in range(2)]
    for coh in range(2):
        for i in range(BPC):
            dmp = p["dump"].tile([P, 64], F32, tag="dump")
            nc.scalar.activation(dmp[:],
                                 h3c[coh][:, i * 64:(i + 1) * 64].bitcast(F32),
                                 ACTF.Relu, bias=0.0, scale=1.0,
                                 accum_out=hf[coh][:, i:i + 1])

    linT = p["fix"].tile([P, 2], F32, tag="linT")
    nc.sync.dma_start(out=linT[:, 0:1], in_=D["linT"][0:128, :])
    nc.sync.dma_start(out=linT[:, 1:2], in_=D["linT"][128:256, :])
    isl = emit_sigma(cx, linT, 256, 1, "lin")
    embT = p["fix"].tile([P, 200], F32, tag="embT")
    nc.sync.dma_start(out=embT[:, 0:100], in_=D["embT"][0:128, :])
    nc.sync.dma_start(out=embT[:, 100:200], in_=D["embT"][128:256, :])
    ise = emit_sigma(cx, embT, 256, 100, "emb")

    y_sb = p["fix"].tile([1, BPC], F32, tag="y_sb")
    nc.sync.dma_start(out=y_sb[:], in_=D["y8"][:, :])
    iot = p["fix"].tile([100, 1], F32, tag="iot")
    nc.sync.dma_start(out=iot[:], in_=D["iota100"][:, :])
    ybc_ps = cx.ps_misc[0:100, 0:BPC]
    nc.tensor.matmul(ybc_ps, cx.ones_row[0:1, 0:100].bitcast(F32), y_sb[:],
                     start=True, stop=True)
    oh = p["fix"].tile([100, BPC], F32, tag="oh")
    nc.vector.tensor_scalar(oh[:], ybc_ps, iot[:], None, ALU.is_equal)
    emb_sb = p["fix"].tile([100, 256], F32, tag="emb_sb")
    nc.sync.dma_start(out=emb_sb[:], in_=D["emb_nat"][:, :])

    Fq = []
    for coh in range(2):
        es_ps = cx.ps_misc[0:P, 0:BPC]
        nc.tensor.matmul(es_ps, emb_sb[:, 128 * coh:128 * coh + 128], oh[:],
                         start=True, stop=True)
        linsc = p["fix"].tile([P, 1], F32, tag=cx.tag("linsc"))
        nc.vector.tensor_scalar(linsc[:], linT[:, coh:coh + 1],
                                isl[:], None, ALU.mult)
        A_sb = p["fix"].tile([P, BPC], F32, tag=cx.tag("A_sb"))
        nc.scalar.activation(A_sb[:], es_ps, ACTF.Identity,
                             bias=linsc[:], scale=ise[:])
        f = p["fix"].tile([P, BPC], F32, tag=cx.tag("Fq"))
        nc.vector.tensor_tensor(f[:], hf[coh][:], A_sb[:], ALU.mult)
        Fq.append(f)
    out_ps = cx.ps_misc[0:1, 0:BPC]
    for coh in range(2):
        nc.tensor.matmul(out_ps, cx.ones_col.bitcast(F32), Fq[coh][:],
                         start=(coh == 0), stop=(coh == 1))
    lb = p["fix"].tile([1, 1], F32, tag="lb")
    nc.sync.dma_start(out=lb[:], in_=D["lin_b"][:, :])
    o_sb = p["fix"].tile([1, BPC], F32, tag="o_sb")
    nc.scalar.activation(o_sb[:], out_ps, ACTF.Identity, bias=lb[:], scale=1.0)
    nc.sync.dma_start(out=o_out[:, :], in_=o_sb[:])

    q_sb = p["fix"].tile([1, BPC], F32, tag="q_sb")
    nc.scalar.copy(q_sb[:], qd_ps[:])
    nc.sync.dma_start(out=o_quant[:, :], in_=q_sb[:])
    nc.sync.dma_start(out=o_counts[:, :], in_=c_acc[:])

    if o_dbg is not None:
        off = 0
        for t, w in ((h0c[0], 2048), (h0c[1], 2048), (h1c[0], 512), (h1c[1], 512),
                     (h2c[0], 512), (h2c[1], 512), (h3c[0], 512), (h3c[1], 512),
                     (hf[0], 8), (hf[1], 8)):
            nc.sync.dma_start(out=o_dbg[:, off:off + w], in_=t[:].bitcast(F32))
            off += w


# ----------------------------------------------------------------- host side

_NC_CACHE = {}


def _get_nc(dev=0, parts=4):
    key = (dev, parts)
    if key not in _NC_CACHE:
        _NC_CACHE[key] = build_module(dev, parts)
    return _NC_CACHE[key]


def _wt3x3(w):
    co, ci = w.shape[0], w.shape[1]
    return _r32r(np.ascontiguousarray(
        np.asarray(w).reshape(co, ci, 3, 3).transpose(2, 3, 1, 0).reshape(
            9 * ci, co)))


def prep_core_inputs(inputs, core):
    d = {}
    s = slice(core * BPC, (core + 1) * BPC)
    d["x8"] = _r32r(inputs["x"][s])
    w1b0 = np.asarray(inputs["b0_w1"]).reshape(256, 3, 3, 3).transpose(2, 3, 1, 0)
    w1b0 = w1b0.reshape(9, 3, 256)[TAP_ORDER].reshape(27, 256)
    d["w1t_b0"] = _r32r(np.ascontiguousarray(w1b0))
    d["w2t_b0"] = _wt3x3(inputs["b0_w2"])
    d["w1t_b1"] = _wt3x3(inputs["b1_w1"])
    d["w2t_b1"] = _wt3x3(inputs["b1_w2"])
    d["w1t_b2"] = _wt3x3(inputs["b2_w1"])
    d["w2t_b2"] = _wt3x3(inputs["b2_w2"])
    d["w1t_b3"] = _wt3x3(inputs["b3_w1"])
    d["w2t_b3"] = _wt3x3(inputs["b3_w2"])
    d["wsct_b0"] = _r32r(np.asarray(inputs["b0_wsc"]).reshape(256, 3).T)
    d["wsct_b1"] = _r32r(np.asarray(inputs["b1_wsc"]).reshape(256, 256).T)
    biases = np.stack([
        inputs["b0_b1"], inputs["b0_b2"], inputs["b0_bsc"],
        inputs["b1_b1"], inputs["b1_b2"], inputs["b1_bsc"],
        inputs["b2_b1"], inputs["b2_b2"],
        inputs["b3_b1"], inputs["b3_b2"]], axis=1).astype(np.float32)
    d["biases"] = np.ascontiguousarray(biases)
    for i in range(4):
        d[f"cb{i}"] = _r32r(inputs[f"cb{i}"])
    d["linT"] = _r32r(np.asarray(inputs["lin_w"]).reshape(1, 256).T)
    d["lin_b"] = np.asarray(inputs["lin_b"], np.float32).reshape(1, 1)
    d["embT"] = _r32r(np.asarray(inputs["emb_w"]).T)
    d["emb_nat"] = _r32r(inputs["emb_w"])
    d["y8"] = _r32r(np.asarray(inputs["y"][s], np.float32).reshape(1, BPC))
    d["iota100"] = np.arange(100, dtype=np.float32).reshape(100, 1)
    cc = np.zeros((P, 8), np.float32)
    cc[:, 0] = 1.0
    cc[:, 1] = C0
    cc[:, 2] = -2.0 * C0
    cc[:, 3] = C1
    cc[:, 4] = -2.0 * C1
    cc[0:64, 6] = -2.0 * C1
    cc[64:128, 7] = -2.0 * C1
    d["cc"] = _r32r(cc)
    d["ones_row_d"] = np.ones((1, P), np.float32)
    d["zpad"] = np.zeros((P, 2592), np.float32)
    return d


def run_cores(inputs, dev=0, **kw):
    nc = _get_nc(dev)
    in_maps = [prep_core_inputs(inputs, c) for c in range(NCORES)]
    return run_bass_kernel_spmd(nc, in_maps, core_ids=list(range(NCORES)), **kw)


def assemble(results):
    outs = np.concatenate([r["o_out"][0] for r in results]).reshape(64, 1)
    quant = np.concatenate([r["o_quant"][0] for r in results]).reshape(64, 1)
    counts = np.sum([r["o_counts"][0] for r in results], axis=0).astype(np.float32)
    probs = counts / np.float32(64 * 8 * 8)
    ppl = np.exp(-np.sum(probs * np.log(probs + np.float32(1e-10)),
                         dtype=np.float32)).astype(np.float32)
    return (outs.astype(np.float32), quant.astype(np.float32), ppl)


def kernel(**inputs):
    inputs = {k: np.asarray(v) for k, v in inputs.items()}
    res = run_cores(inputs)
    return assemble(res.results)


# revision 22
# speedup vs baseline: 1.3111x; 1.0660x over previous
"""Trainium2 Bass kernel for nn_Discriminator_61332132987171 (vq_codebook).

Data-parallel over batch: 8 images per NeuronCore across 8 cores.
All matmuls in float32r (fp32 with 11-bit mantissa, full PE rate at N>=256).

Per-core pipeline:
  block0: im2col conv 3->256 (K=27), conv 256->256 (9-tap accumulation over a
          zero-padded SBUF plane), avgpool2, 1x1-conv shortcut on pooled x
  VQ0..VQ3: T[pix,code] = x.e - |e|^2/2 via matmuls (codes on free dim),
          max-reduce over codes, per-image sums into one PSUM accumulator
  block1: preact block at 16x16 with downsample + 1x1 shortcut
  block2/3: preact blocks at 8x8, image-batched matmuls (3D moving APs)
  head: hf = sum relu(h3) via ACT accum_out; out = hf . (lin/sl + emb[y]/se) + b
  spectral-norm 1/sigma for every weight computed on device and folded into
  the PSUM-eviction activation scale.

Host side only shards/transposes/rounds inputs and reduces per-core histogram
counts into the final perplexity scalar.
"""
import sys

for _p in ("/opt/trn_rl_repo", "/opt/pypackages"):
    if _p not in sys.path:
        sys.path.append(_p)

import numpy as np
import concourse.bass as bass  # noqa: F401
import concourse.mybir as mybir
import concourse.tile as tile
from concourse import bacc
from concourse.bass_utils import run_bass_kernel_spmd

F32 = mybir.dt.float32
F32R = mybir.dt.float32r
AX = mybir.AxisListType
ALU = mybir.AluOpType
ACTF = mybir.ActivationFunctionType

P = 128
NCORES = 8
BPC = 8  # images per core

C0 = 0.5 / (256.0 * 16 * 16)   # quant-loss scale, block0
C1 = 0.5 / (256.0 * 8 * 8)     # blocks 1-3
TAP_ORDER = [4, 0, 1, 2, 3, 5, 6, 7, 8]  # center tap first (partitions 0..2)


def _r32r(x):
    """Round fp32 -> fp32r (11-bit mantissa, RTNE) on host."""
    u = np.ascontiguousarray(x, dtype=np.float32).view(np.uint32)
    u2 = u + 0x7FF + ((u >> 12) & 1)
    return (u2 & 0xFFFFF000).astype(np.uint32).view(np.float32)


class Ctx:
    def __init__(self, nc, tc, pools, inp):
        self.nc = nc
        self.tc = tc
        self.p = pools
        self.inp = inp
        self.uid = 0
        self.qd_first = True

    def tag(self, base):
        self.uid += 1
        return f"{base}{self.uid}"


def emit_bcast11(cx, src11_f32_ap, tagbase):
    """[1,1] f32 AP -> [128,1] f32 SBUF column (K=1 ones matmul broadcast)."""
    nc, p = cx.nc, cx.p
    s_r = p["fix"].tile([1, 1], F32, tag=cx.tag("bc_r"))
    nc.vector.tensor_copy(s_r[:], src11_f32_ap)
    bc_ps = cx.ps_misc[0:P, 0:1]
    nc.tensor.matmul(bc_ps, cx.ones_row[0:1, 0:P].bitcast(F32), s_r[:],
                     start=True, stop=True)
    col = p["fix"].tile([P, 1], F32, tag=cx.tag(tagbase))
    nc.vector.tensor_copy(col[:], bc_ps)
    return col


def emit_sigma(cx, wt_sb, K, M, name):
    """1/sigma (spectral norm, 1 power iter) for WT layout [K, M-per-ktile]:
    k-tile j lives at wt_sb[:, j*M:(j+1)*M].  Returns [128,1] f32 column."""
    nc, p = cx.nc, cx.p
    kt = (K + P - 1) // P
    mh = (M + P - 1) // P
    kp = min(P, K)

    vtmp = p["fix"].tile([kp, kt], F32, tag=cx.tag("sg_vt"))
    for j in range(kt):
        pt = min(P, K - P * j)
        nc.vector.tensor_reduce(vtmp[0:pt, j:j + 1], wt_sb[0:pt, j * M:(j + 1) * M],
                                axis=AX.X, op=ALU.add)
    vcol = p["fix"].tile([kp, kt], F32, tag=cx.tag("sg_vc"))
    nc.vector.tensor_scalar(vcol[:], vtmp[:], 1.0 / float(np.sqrt(M)), None, ALU.mult)

    s_ps = cx.ps_misc[0:1, 0:1]
    for j in range(kt):
        pt = min(P, K - P * j)
        nc.tensor.matmul(s_ps, vcol[0:pt, j:j + 1], vcol[0:pt, j:j + 1],
                         start=(j == 0), stop=(j == kt - 1))
    s_sb = p["fix"].tile([1, 1], F32, tag=cx.tag("sg_s"))
    nc.vector.tensor_copy(s_sb[:], s_ps)
    a_sb = p["fix"].tile([1, 1], F32, tag=cx.tag("sg_a"))
    nc.scalar.sqrt(a_sb[:], s_sb[:])
    d_sb = p["fix"].tile([1, 1], F32, tag=cx.tag("sg_d"))
    nc.vector.tensor_scalar(d_sb[:], a_sb[:], 1e-8, None, ALU.add)
    r_sb = p["fix"].tile([1, 1], F32, tag=cx.tag("sg_r"))
    nc.vector.reciprocal(r_sb[:], d_sb[:])
    rcol = emit_bcast11(cx, r_sb[:], "sg_rc")

    vhat = p["fix"].tile([kp, kt], F32, tag=cx.tag("sg_vh"))
    nc.vector.tensor_scalar(vhat[:], vcol[:], rcol[0:kp, :], None, ALU.mult)

    wv_ps = cx.ps_misc[0:P, 0:mh]
    for h in range(mh):
        mw = min(P, M - P * h)
        for j in range(kt):
            pt = min(P, K - P * j)
            nc.tensor.matmul(
                cx.ps_misc[0:mw, h:h + 1],
                wt_sb[0:pt, j * M + h * P: j * M + h * P + mw].bitcast(F32),
                vhat[0:pt, j:j + 1],
                start=(h == 0 and j == 0), stop=(h == mh - 1 and j == kt - 1))
    wv_sb = p["fix"].tile([P, mh], F32, tag=cx.tag("sg_wv"))
    for h in range(mh):
        mw = min(P, M - P * h)
        nc.vector.tensor_copy(wv_sb[0:mw, h:h + 1], cx.ps_misc[0:mw, h:h + 1])

    s2_ps = cx.ps_misc[0:1, 0:1]
    for h in range(mh):
        mw = min(P, M - P * h)
        nc.tensor.matmul(s2_ps, wv_sb[0:mw, h:h + 1], wv_sb[0:mw, h:h + 1],
                         start=(h == 0), stop=(h == mh - 1))
    s2_sb = p["fix"].tile([1, 1], F32, tag=cx.tag("sg_s2"))
    nc.vector.tensor_copy(s2_sb[:], s2_ps)
    a2 = p["fix"].tile([1, 1], F32, tag=cx.tag("sg_a2"))
    nc.scalar.sqrt(a2[:], s2_sb[:])
    d2 = p["fix"].tile([1, 1], F32, tag=cx.tag("sg_d2"))
    nc.vector.tensor_scalar(d2[:], a2[:], 1e-8, None, ALU.add)
    r2 = p["fix"].tile([1, 1], F32, tag=cx.tag("sg_r2"))
    nc.vector.reciprocal(r2[:], s2_sb[:])
    inv11 = p["fix"].tile([1, 1], F32, tag=cx.tag("sg_i"))
    nc.vector.tensor_mul(inv11[:], d2[:], r2[:])
    return emit_bcast11(cx, inv11[:], f"invs_{name}")


def emit_load_weight(cx, dram, K, M, name):
    """DMA WT [K, M] -> SBUF k-tiles along free dim; compute 1/sigma."""
    nc, p = cx.nc, cx.p
    kt = (K + P - 1) // P
    if K > 256:
        wt_sb = p["wbig"].tile([P, kt * M], F32R, tag="wbig")
    else:
        wt_sb = p["fix"].tile([min(P, K), kt * M], F32R, tag=cx.tag(f"w_{name}"))
    for j in range(kt):
        pt = min(P, K - P * j)
        nc.sync.dma_start(out=wt_sb[0:pt, j * M:(j + 1) * M],
                          in_=dram[j * P: j * P + pt, :])
    invs = emit_sigma(cx, wt_sb, K, M, name)
    return wt_sb, invs


def emit_load_cb(cx, dram):
    """Codebook [256,1024] -> (cbA, cbB, e2neg) with e2neg = -|e_k|^2/2."""
    nc, p = cx.nc, cx.p
    cbA = p["cbA"].tile([P, 1024], F32R, tag="cbA")
    cbB = p["cbB"].tile([P, 1024], F32R, tag="cbB")
    nc.sync.dma_start(out=cbA[:], in_=dram[0:128, :])
    nc.sync.dma_start(out=cbB[:], in_=dram[128:256, :])
    e2neg = p["e2"].tile([1, 1024], F32R, tag="e2")
    for kh in range(2):
        ksl = slice(512 * kh, 512 * kh + 512)
        sq = p["tsb"].tile([P, 1024], F32R, tag="tsb")
        nc.vector.tensor_tensor(sq[:, 0:512], cbA[:, ksl].bitcast(F32),
                                cbA[:, ksl].bitcast(F32), ALU.mult)
        nc.vector.tensor_tensor(sq[:, 512:1024], cbB[:, ksl].bitcast(F32),
                                cbB[:, ksl].bitcast(F32), ALU.mult)
        e2_ps = cx.ps_misc[0:1, 0:512]
        nc.tensor.matmul(e2_ps, cx.ones_col, sq[:, 0:512], start=True, stop=False)
        nc.tensor.matmul(e2_ps, cx.ones_col, sq[:, 512:1024], start=False, stop=True)
        nc.scalar.activation(e2neg[0:1, ksl], e2_ps, ACTF.Copy,
                             bias=0.0, scale=-0.5)
    return cbA, cbB, e2neg


def emit_vq(cx, h_cmp, cbA, cbB, e2neg, n_pix, pix_per_img, c_scale, qd_ps,
            c_acc, qd_close=False, vq_mode="full", imgs=None):
    """VQ pass over compact activations h_cmp = 2 x [128, n_pix] (f32r).

    T[pix, code] = x.e - |e|^2/2;  min_dist = |x|^2 - 2*max_code(T).
    Accumulates c*sum(h^2) - 2c*sum(Tmax) per image into qd_ps[0, img].
    If c_acc is not None, adds code histogram counts into it ([1,1024] f32).
    """
    nc, p = cx.nc, cx.p
    ntile = n_pix // P
    n_img = n_pix // pix_per_img
    img_per_tile = max(1, P // pix_per_img)
    tile_per_img = max(1, pix_per_img // P)
    cpos = cx.cpos0 if c_scale == C0 else cx.cpos1
    cneg = cx.cneg0 if c_scale == C0 else cx.cneg1
    if imgs is None:
        imgs = range(n_img)
    tiles = sorted({(i * tile_per_img + k) // img_per_tile
                    for i in imgs for k in range(tile_per_img)})

    for i in imgs:
        if vq_mode == "T":
            break
        for coh in range(2):
            hsl = h_cmp[coh][:, i * pix_per_img:(i + 1) * pix_per_img].bitcast(F32)
            dump = p["dump"].tile([P, pix_per_img], F32, tag="dump")
            sqa = p["cols"].tile([P, 1], F32, tag="cols")
            nc.vector.tensor_tensor(dump[:], hsl, hsl, ALU.mult)
            nc.vector.tensor_reduce(sqa[:], dump[:], axis=AX.X, op=ALU.add)
            if vq_mode == "ttr":
                nc.sync.dma_start(out=cx.dbg_T[:, 2 * i + coh: 2 * i + coh + 1],
                                  in_=sqa[:])
                continue
            nc.tensor.matmul(qd_ps[0:1, i:i + 1], sqa[:], cpos.bitcast(F32),
                             start=cx.qd_first,
                             stop=(vq_mode == "sq" and i == n_img - 1 and coh == 1))
            cx.qd_first = False

    if vq_mode in ("sq", "ttr"):
        return
    for t in tiles:
        pix0 = t * P
        T_sb = p["tsb"].tile([P, 1024], F32, tag="tsb")
        for kh in range(2):
            ksl = slice(512 * kh, 512 * kh + 512)
            T_ps = p["vq"].tile([P, 512], F32, tag="vq")
            nc.tensor.matmul(T_ps[:], h_cmp[0][:, pix0:pix0 + P], cbA[:, ksl],
                             start=True, stop=False)
            nc.tensor.matmul(T_ps[:], h_cmp[1][:, pix0:pix0 + P], cbB[:, ksl],
                             start=False, stop=False)
            nc.tensor.matmul(T_ps[:], cx.ones_row[0:1, 0:P], e2neg[0:1, ksl],
                             start=False, stop=True)
            nc.scalar.copy(T_sb[:, ksl], T_ps[:])

        tmx = p["cols"].tile([P, 1], F32, tag="cols")
        nc.vector.tensor_reduce(tmx[:], T_sb[:], axis=AX.X, op=ALU.max)
        if vq_mode == "T":
            nc.sync.dma_start(out=cx.dbg_T[:, t:t + 1], in_=tmx[:])
            continue
        last = qd_close and t == ntile - 1
        if img_per_tile <= 1:
            i = t // tile_per_img
            nc.tensor.matmul(qd_ps[0:1, i:i + 1], tmx[:], cneg.bitcast(F32),
                             start=False, stop=last)
        else:
            assert img_per_tile == 2
            i0 = t * 2
            nc.tensor.matmul(qd_ps[0:1, i0:i0 + 2], tmx[:],
                             cx.ind2.bitcast(F32), start=False, stop=last)

        if c_acc is not None:
            eq = p["tsb"].tile([P, 1024], F32R, tag="tsb")
            nc.vector.tensor_scalar(eq[:], T_sb[:], tmx[:], None, ALU.is_equal)
            for kh in range(2):
                cnt_ps = p["sc"].tile([1, 512], F32, tag="conv")
                nc.tensor.matmul(cnt_ps[:], cx.ones_col,
                                 eq[:, 512 * kh:512 * kh + 512],
                                 start=True, stop=True)
                nc.vector.tensor_tensor(c_acc[0:1, 512 * kh:512 * kh + 512],
                                        c_acc[0:1, 512 * kh:512 * kh + 512],
                                        cnt_ps[:], ALU.add)


def emit_pool2(cx, src_ap, rows, cols):
    """2x2 sum-pool of SBUF tensor [128, rows*cols] -> dump tile [128, rows*cols/4]."""
    nc, p = cx.nc, cx.p
    hc = cols // 2
    hr = rows // 2
    t1 = p["dump"].tile([P, rows * hc], F32, tag="dump")
    v = src_ap.rearrange("p (r c two) -> p r c two", r=rows, c=hc, two=2)
    nc.vector.tensor_tensor(t1[:], v[:, :, :, 0], v[:, :, :, 1], ALU.add)
    t1v = t1[:].rearrange("p (r two c) -> p r two c", r=hr, two=2, c=hc)
    t2 = p["dump"].tile([P, hr * hc], F32, tag="dump")
    nc.vector.tensor_tensor(t2[:], t1v[:, :, 0, :], t1v[:, :, 1, :], ALU.add)
    return t2


def emit_pool_fuse(cx, y_ps_ap, sc_sb_ap, h_out_ap, rows, cols, scale_col,
                   bsum_col):
    """h_out = poolsum2x2(y_ps * scale_col) + bsum_col + sc_sb."""
    nc, p = cx.nc, cx.p
    y_sb = p["dump"].tile([P, rows * cols], F32, tag="dump")
    nc.scalar.activation(y_sb[:], y_ps_ap, ACTF.Identity, bias=0.0,
                         scale=scale_col[:])
    t2 = emit_pool2(cx, y_sb[:], rows, cols)
    nc.vector.scalar_tensor_tensor(h_out_ap, t2[:], bsum_col, sc_sb_ap,
                                   op0=ALU.add, op1=ALU.add)


DBG_WIDTH = 4096 + 1024 + 1024 + 1024 + 16


def build_module(dev=0, parts=4):
    nc = bacc.Bacc("TRN2", target_bir_lowering=False, debug=False)
    D = {}

    def din(name, shape, dt=F32R):
        D[name] = nc.dram_tensor(name, shape, dt, kind="ExternalInput")

    din("x8", [BPC, 3, 32, 32])
    din("w1t_b0", [27, 256])
    for nm in ("w2t_b0", "w1t_b1", "w2t_b1", "w1t_b2", "w2t_b2", "w1t_b3",
               "w2t_b3"):
        din(nm, [2304, 256])
    din("wsct_b0", [3, 256])
    din("wsct_b1", [256, 256])
    din("biases", [256, 10], F32)
    for i in range(4):
        din(f"cb{i}", [256, 1024])
    din("linT", [256, 1], F32)
    din("lin_b", [1, 1], F32)
    din("embT", [256, 100], F32)
    din("emb_nat", [100, 256], F32)
    din("y8", [1, BPC], F32)
    din("iota100", [100, 1], F32)
    din("cc", [P, 8])
    din("ones_row_d", [1, P])
    din("zpad", [P, 2592])

    o_out = nc.dram_tensor("o_out", [1, BPC], F32, kind="ExternalOutput")
    o_quant = nc.dram_tensor("o_quant", [1, BPC], F32, kind="ExternalOutput")
    o_counts = nc.dram_tensor("o_counts", [1, 1024], F32, kind="ExternalOutput")
    o_dbg = None
    if dev:
        o_dbg = nc.dram_tensor("o_dbg", [P, DBG_WIDTH], F32, kind="ExternalOutput")

    with tile.TileContext(nc) as tc:
        with (
            tc.tile_pool(name="fix", bufs=1) as fix,
            tc.tile_pool(name="wbig", bufs=2) as wbig,
            tc.tile_pool(name="cbA", bufs=2) as cbA_p,
            tc.tile_pool(name="cbB", bufs=2) as cbB_p,
            tc.tile_pool(name="e2", bufs=2) as e2_p,
            tc.tile_pool(name="tsb", bufs=2) as tsb,
            tc.tile_pool(name="dump", bufs=6) as dump,
            tc.tile_pool(name="cols", bufs=8) as cols,
            tc.tile_pool(name="conv", bufs=4, space="PSUM") as conv_ps,
            tc.tile_pool(name="vq", bufs=2, space="PSUM") as vq_psp,
            tc.tile_pool(name="qd", bufs=1, space="PSUM") as qd_p,
            tc.tile_pool(name="misc", bufs=1, space="PSUM") as misc,
        ):
            pools = dict(fix=fix, wbig=wbig, cbA=cbA_p, cbB=cbB_p, e2=e2_p,
                         tsb=tsb, dump=dump, cols=cols, conv=conv_ps, sc=conv_ps,
                         vq=vq_psp, qd=qd_p, misc=misc)
            cx = Ctx(nc, tc, pools, D)
            emit_all(cx, D, o_out, o_quant, o_counts, o_dbg, parts)

    nc.compile()
    return nc


def emit_all(cx, D, o_out, o_quant, o_counts, o_dbg, parts=4):
    nc, p = cx.nc, cx.p

    # ---- consts / biases
    cc = p["fix"].tile([P, 8], F32R, tag="cc")
    nc.sync.dma_start(out=cc[:], in_=D["cc"][:, :])
    cx.ones_col = cc[:, 0:1]
    cx.cpos0 = cc[:, 1:2]
    cx.cneg0 = cc[:, 2:3]
    cx.cpos1 = cc[:, 3:4]
    cx.cneg1 = cc[:, 4:5]
    cx.ind2 = cc[:, 6:8]
    ones_row = p["fix"].tile([1, P], F32R, tag="ones_row")
    nc.sync.dma_start(out=ones_row[:], in_=D["ones_row_d"][:, :])
    cx.ones_row = ones_row

    bias = p["fix"].tile([P, 20], F32, tag="bias")
    nc.sync.dma_start(out=bias[:, 0:10], in_=D["biases"][0:128, :])
    nc.sync.dma_start(out=bias[:, 10:20], in_=D["biases"][128:256, :])

    def bcol(idx, coh):
        return bias[:, 10 * coh + idx: 10 * coh + idx + 1]

    bsum = p["fix"].tile([P, 4], F32, tag="bsum")
    nc.vector.tensor_tensor(bsum[:, 0:1], bcol(2, 0), bcol(1, 0), ALU.add)
    nc.vector.tensor_tensor(bsum[:, 1:2], bcol(2, 1), bcol(1, 1), ALU.add)
    nc.vector.tensor_tensor(bsum[:, 2:3], bcol(5, 0), bcol(4, 0), ALU.add)
    nc.vector.tensor_tensor(bsum[:, 3:4], bcol(5, 1), bcol(4, 1), ALU.add)

    qd_full = p["qd"].tile([1, 512], F32, tag="qd")
    qd_ps = qd_full[0:1, 0:BPC]
    cx.ps_misc = p["misc"].tile([P, 512], F32, tag="misc")
    c_acc = p["fix"].tile([1, 1024], F32, tag="c_acc")
    nc.vector.memset(c_acc[:], 0.0)

    # ---- block0 weights
    w1, is1 = emit_load_weight(cx, D["w1t_b0"], 27, 256, "b0w1")
    w2, is2 = emit_load_weight(cx, D["w2t_b0"], 2304, 256, "b0w2")
    wsc, issc = emit_load_weight(cx, D["wsct_b0"], 3, 256, "b0wsc")
    cb0A, cb0B, e2n0 = emit_load_cb(cx, D["cb0"])

    def scale4(col, nm):
        out = p["fix"].tile([P, 1], F32, tag=cx.tag(nm))
        nc.vector.tensor_scalar(out[:], col[:], 0.25, None, ALU.mult)
        return out

    is2_4 = scale4(is2, "is2_4")
    issc_4 = scale4(issc, "issc_4")

    # im2col buffers, zeroed once (tap-valid regions identical per image)
    X0 = []
    for b in range(2):
        t = p["fix"].tile([27, 1024], F32R, tag=f"X0_{b}")
        nc.sync.dma_start(out=t[:], in_=D["zpad"][0:27, 0:1024])
        X0.append(t)

    def zero_borders(t, side, nimg):
        n = nimg * side * side
        nc.sync.dma_start(out=t[:, 0:n], in_=D["zpad"][:, 0:n])

    def padded_plane(tagbase, side, nimg):
        pair = []
        for coh in range(2):
            t = p["fix"].tile([P, nimg * side * side], F32R,
                              tag=f"{tagbase}_{coh}")
            zero_borders(t, side, nimg)
            pair.append(t)
        return pair

    y1p = []
    for b in range(2):
        pair = []
        for coh in range(2):
            t = p["fix"].tile([P, 34 * 34], F32R, tag=f"y1p_{b}_{coh}")
            zero_borders(t, 34, 1)
            pair.append(t)
        y1p.append(pair)

    rh0 = padded_plane("rh0", 18, BPC)
    h0c = [p["fix"].tile([P, BPC * 256], F32R, tag=f"h0c_{c}", name=f"h0c_{c}") for c in range(2)]

    # ---------------- block 0 ----------------
    if parts < 0.2:
        if o_dbg is not None:
            nc.sync.dma_start(out=o_dbg[:, 0:1], in_=is1[:])
            nc.sync.dma_start(out=o_dbg[:, 1:2], in_=is2[:])
            nc.sync.dma_start(out=o_dbg[:, 2:3], in_=issc[:])
            nc.sync.dma_start(out=o_dbg[0:1, 16:1040], in_=e2n0[:].bitcast(F32))
        return
    dma_eng = [nc.sync, nc.sync, nc.sync]
    for i in range(BPC):
        xb = X0[i % 2]
        dstv = xb[:].rearrange("p (r c) -> p r c", r=32, c=32)
        for pos, t9 in enumerate(TAP_ORDER):
            dy, dx = t9 // 3, t9 % 3
            hlo, hhi = max(0, 1 - dy), min(32, 33 - dy)
            wlo, whi = max(0, 1 - dx), min(32, 33 - dx)
            dma_eng[pos % 3].dma_start(
                out=dstv[3 * pos:3 * pos + 3, hlo:hhi, wlo:whi],
                in_=D["x8"][i, :, hlo + dy - 1:hhi + dy - 1,
                            wlo + dx - 1:whi + dx - 1])
        yb = y1p[i % 2]
        if parts < 0.4:
            if o_dbg is not None:
                nc.sync.dma_start(out=o_dbg[0:27, 1040 + i * 128:1040 + i * 128 + 128],
                                  in_=xb[:, 0:128].bitcast(F32))
            continue
        for s in range(2):
            for coh in range(2):
                ps = p["conv"].tile([P, 512], F32, tag="conv")
                nc.tensor.matmul(ps[:], w1[0:27, 128 * coh:128 * coh + 128],
                                 xb[:, 512 * s:512 * s + 512], start=True, stop=True)
                ov = yb[coh][:].rearrange("p (r c) -> p r c", r=34, c=34)
                nc.scalar.activation(ov[:, 1 + 16 * s:17 + 16 * s, 1:33], ps[:],
                                     ACTF.Relu, bias=bcol(0, coh), scale=is1[:])
        if parts < 0.6:
            if o_dbg is not None:
                nc.sync.dma_start(out=o_dbg[:, 1040 + i * 128:1040 + i * 128 + 128],
                                  in_=yb[0][:, 0:128].bitcast(F32))
            continue
        # pooled x (sum form) from im2col center tap (rows 12..14 = full image)
        xt1 = p["dump"].tile([3, 512], F32, tag="dump")
        xcv = xb[0:3, :].rearrange("p (r c two) -> p r c two", r=32, c=16, two=2)
        nc.vector.tensor_tensor(xt1[:], xcv[:, :, :, 0], xcv[:, :, :, 1], ALU.add)
        xp_i = p["dump"].tile([3, 256], F32R, tag="dump")
        xt1v = xt1[:].rearrange("p (r two c) -> p r two c", r=16, two=2, c=16)
        nc.vector.tensor_tensor(xp_i[:], xt1v[:, :, 0, :], xt1v[:, :, 1, :], ALU.add)
        sc_psT = p["sc"].tile([P, 512], F32, tag="conv")
        for coh in range(2):
            nc.tensor.matmul(sc_psT[:, 256 * coh:256 * coh + 256],
                             wsc[0:3, 128 * coh:128 * coh + 128],
                             xp_i[:], start=(coh == 0), stop=(coh == 1))
        for s in range(2):
            c2 = []
            for coh in range(2):
                ps = p["conv"].tile([P, 512], F32, tag="conv")
                first = True
                for t9 in range(9):
                    dy, dx = t9 // 3, t9 % 3
                    for cih in range(2):
                        src = yb[cih][:].rearrange("p (r c) -> p r c", r=34, c=34)
                        rhs = src[:, dy + 16 * s: dy + 16 * s + 16, dx:dx + 32]
                        nc.tensor.matmul(
                            ps[:],
                            w2[:, (2 * t9 + cih) * 256 + 128 * coh:
                                  (2 * t9 + cih) * 256 + 128 * coh + 128],
                            rhs, start=first, stop=(t9 == 8 and cih == 1))
                        first = False
                c2.append(ps)
            for coh in range(2):
                sc_sb = p["dump"].tile([P, 128], F32, tag="dump")
                scv = sc_psT[:, 256 * coh:256 * coh + 256].rearrange(
                    "p (r c) -> p r c", r=16, c=16)
                nc.scalar.activation(
                    sc_sb[:].rearrange("p (r c) -> p r c", r=8, c=16),
                    scv[:, 8 * s:8 * s + 8, :],
                    ACTF.Identity, bias=0.0, scale=issc_4[:])
                emit_pool_fuse(
                    cx, c2[coh][:], sc_sb[:],
                    h0c[coh][:, i * 256 + 128 * s: i * 256 + 128 * s + 128],
                    16, 32, is2_4, bsum[:, coh:coh + 1])
        for coh in range(2):
            ov = rh0[coh][:].rearrange("p (i r c) -> p i r c", i=BPC, r=18, c=18)
            nc.scalar.activation(
                ov[:, i, 1:17, 1:17],
                h0c[coh][:, i * 256:(i + 1) * 256].bitcast(F32).rearrange(
                    "p (r c) -> p r c", r=16, c=16),
                ACTF.Relu, bias=0.0, scale=1.0)
        if parts >= 1 and parts not in (0.8, 0.85, 0.9):
            emit_vq(cx, h0c, cb0A, cb0B, e2n0, BPC * 256, 256, C0, qd_ps, None,
                    qd_close=False, imgs=[i])

    if parts in (0.8, 0.85, 0.9):
        pass
    elif parts < 1:
        if o_dbg is not None and parts >= 0.6:
            for t, w, off in ((h0c[0], 2048, 0), (h0c[1], 2048, 2048)):
                nc.sync.dma_start(out=o_dbg[:, off:off + w], in_=t[:].bitcast(F32))
        return
    if parts in (0.8, 0.85, 0.9):
        cx.dbg_T = o_dbg
        emit_vq(cx, h0c, cb0A, cb0B, e2n0, BPC * 256, 256, C0, qd_ps, None,
                qd_close=False,
                vq_mode={0.8: "sq", 0.85: "ttr", 0.9: "T"}[parts])
        if parts == 0.8:
            q_sb = p["fix"].tile([1, BPC], F32, tag="q_sb")
            nc.scalar.copy(q_sb[:], qd_ps)
            nc.sync.dma_start(out=o_quant[:, :], in_=q_sb[:])
        return
    if parts >= 2:
        emit_rest(cx, D, o_out, o_quant, o_counts, o_dbg, parts,
                  bcol, bsum, qd_ps, c_acc, h0c, rh0, padded_plane, scale4,
                  zero_borders)
    else:
        q_sb = p["fix"].tile([1, BPC], F32, tag="q_sb")
        nc.scalar.copy(q_sb[:], qd_ps)
        nc.sync.dma_start(out=o_quant[:, :], in_=q_sb[:])
        if o_dbg is not None:
            for t, w, off in ((h0c[0], 2048, 0), (h0c[1], 2048, 2048)):
                nc.sync.dma_start(out=o_dbg[:, off:off + w], in_=t[:].bitcast(F32))


def emit_rest(cx, D, o_out, o_quant, o_counts, o_dbg, parts,
              bcol, bsum, qd_ps, c_acc, h0c, rh0, padded_plane, scale4,
              zero_borders):
    nc, p = cx.nc, cx.p
    # ---------------- block 1 ----------------
    w1b1, is1b1 = emit_load_weight(cx, D["w1t_b1"], 2304, 256, "b1w1")
    w2b1, is2b1 = emit_load_weight(cx, D["w2t_b1"], 2304, 256, "b1w2")
    wscb1, isscb1 = emit_load_weight(cx, D["wsct_b1"], 256, 256, "b1wsc")
    cb1A, cb1B, e2n1 = emit_load_cb(cx, D["cb1"])
    is2b1_4 = scale4(is2b1, "is2b1_4")
    isscb1_4 = scale4(isscb1, "isscb1_4")

    rt1 = []
    for b in range(2):
        pair = []
        for coh in range(2):
            t = p["fix"].tile([P, 18 * 18], F32R, tag=f"rt1_{b}_{coh}")
            zero_borders(t, 18, 1)
            pair.append(t)
        rt1.append(pair)

    rh1 = padded_plane("rh1", 10, BPC)
    h1c = [p["fix"].tile([P, BPC * 64], F32R, tag=f"h1c_{c}", name=f"h1c_{c}") for c in range(2)]

    for i in range(BPC):
        rb = rt1[i % 2]
        for coh in range(2):
            psf = p["conv"].tile([P, 512], F32, tag="conv")
            ps = psf[:, 0:256]
            first = True
            for t9 in range(9):
                dy, dx = t9 // 3, t9 % 3
                for cih in range(2):
                    src = rh0[cih][:].rearrange("p (i r c) -> p i r c",
                                                i=BPC, r=18, c=18)
                    rhs = src[:, i, dy:dy + 16, dx:dx + 16]
                    nc.tensor.matmul(
                        ps[:],
                        w1b1[:, (2 * t9 + cih) * 256 + 128 * coh:
                                (2 * t9 + cih) * 256 + 128 * coh + 128],
                        rhs, start=first, stop=(t9 == 8 and cih == 1))
                    first = False
            ov = rb[coh][:].rearrange("p (r c) -> p r c", r=18, c=18)
            nc.scalar.activation(
                ov[:, 1:17, 1:17],
                ps[:].rearrange("p (r c) -> p r c", r=16, c=16),
                ACTF.Relu, bias=bcol(3, coh), scale=is1b1[:])
        sc_psT = p["sc"].tile([P, 512], F32, tag="conv")
        for coh in range(2):
            for cih in range(2):
                nc.tensor.matmul(
                    sc_psT[:, 256 * coh:256 * coh + 256],
                    wscb1[:, cih * 256 + 128 * coh: cih * 256 + 128 * coh + 128],
                    h0c[cih][:, i * 256:(i + 1) * 256],
                    start=(coh == 0 and cih == 0), stop=(coh == 1 and cih == 1))
        for coh in range(2):
            psf = p["conv"].tile([P, 512], F32, tag="conv")
            ps = psf[:, 0:256]
            first = True
            for t9 in range(9):
                dy, dx = t9 // 3, t9 % 3
                for cih in range(2):
                    src = rb[cih][:].rearrange("p (r c) -> p r c", r=18, c=18)
                    rhs = src[:, dy:dy + 16, dx:dx + 16]
                    nc.tensor.matmul(
                        ps[:],
                        w2b1[:, (2 * t9 + cih) * 256 + 128 * coh:
                                (2 * t9 + cih) * 256 + 128 * coh + 128],
                        rhs, start=first, stop=(t9 == 8 and cih == 1))
                    first = False
            sc_e = p["dump"].tile([P, 256], F32, tag="dump")
            nc.scalar.activation(sc_e[:], sc_psT[:, 256 * coh:256 * coh + 256],
                                 ACTF.Identity, bias=0.0, scale=isscb1_4[:])
            sc_t2 = emit_pool2(cx, sc_e[:], 16, 16)
            emit_pool_fuse(cx, ps[:], sc_t2[:],
                           h1c[coh][:, i * 64:(i + 1) * 64], 16, 16, is2b1_4,
                           bsum[:, 2 + coh:3 + coh])
        for coh in range(2):
            ov = rh1[coh][:].rearrange("p (i r c) -> p i r c", i=BPC, r=10, c=10)
            nc.scalar.activation(
                ov[:, i, 1:9, 1:9],
                h1c[coh][:, i * 64:(i + 1) * 64].bitcast(F32).rearrange(
                    "p (r c) -> p r c", r=8, c=8),
                ACTF.Relu, bias=0.0, scale=1.0)
        if i % 2 == 1:
            emit_vq(cx, h1c, cb1A, cb1B, e2n1, BPC * 64, 64, C1, qd_ps, None,
                    qd_close=False, imgs=[i - 1, i])
    if parts < 3:
        q_sb = p["fix"].tile([1, BPC], F32, tag="q_sb")
        nc.scalar.copy(q_sb[:], qd_ps)
        nc.sync.dma_start(out=o_quant[:, :], in_=q_sb[:])
        if o_dbg is not None:
            dl = ((h0c[0], 2048, 0), (h0c[1], 2048, 2048),
                  (h1c[0], 512, 4096), (h1c[1], 512, 4608))
            for t, w, off in dl:
                nc.sync.dma_start(out=o_dbg[:, off:off + w], in_=t[:].bitcast(F32))
        return

    # ---------------- blocks 2 & 3 ----------------
    def emit_block23(rh_in, h_in, w1d, w2d, cbd, b1i, b2i, nm, c_acc_arg):
        w1x, is1x = emit_load_weight(cx, w1d, 2304, 256, nm + "w1")
        w2x, is2x = emit_load_weight(cx, w2d, 2304, 256, nm + "w2")
        cbXA, cbXB, e2nX = emit_load_cb(cx, cbd)
        rt = padded_plane("rt_" + nm, 10, BPC)
        h_out = [p["fix"].tile([P, BPC * 64], F32R, tag=f"h_{nm}_{c}",
                               name=f"h_{nm}_{c}") for c in range(2)]
        for coh in range(2):
            ps = p["conv"].tile([P, 512], F32, tag="conv")
            first = True
            for t9 in range(9):
                dy, dx = t9 // 3, t9 % 3
                for cih in range(2):
                    src = rh_in[cih][:].rearrange("p (i r c) -> p i r c",
                                                  i=BPC, r=10, c=10)
                    rhs = src[:, :, dy:dy + 8, dx:dx + 8]
                    nc.tensor.matmul(
                        ps[:],
                        w1x[:, (2 * t9 + cih) * 256 + 128 * coh:
                                (2 * t9 + cih) * 256 + 128 * coh + 128],
                        rhs, start=first, stop=(t9 == 8 and cih == 1))
                    first = False
            ov = rt[coh][:].rearrange("p (i r c) -> p i r c", i=BPC, r=10, c=10)
            nc.scalar.activation(
                ov[:, :, 1:9, 1:9],
                ps[:].rearrange("p (i r c) -> p i r c", i=BPC, r=8, c=8),
                ACTF.Relu, bias=bcol(b1i, coh), scale=is1x[:])
        for coh in range(2):
            ps = p["conv"].tile([P, 512], F32, tag="conv")
            first = True
            for t9 in range(9):
                dy, dx = t9 // 3, t9 % 3
                for cih in range(2):
                    src = rt[cih][:].rearrange("p (i r c) -> p i r c",
                                               i=BPC, r=10, c=10)
                    rhs = src[:, :, dy:dy + 8, dx:dx + 8]
                    nc.tensor.matmul(
                        ps[:],
                        w2x[:, (2 * t9 + cih) * 256 + 128 * coh:
                                (2 * t9 + cih) * 256 + 128 * coh + 128],
                        rhs, start=first, stop=(t9 == 8 and cih == 1))
                    first = False
            tmp = p["dump"].tile([P, 512], F32, tag="dump")
            nc.scalar.activation(tmp[:], ps[:], ACTF.Identity,
                                 bias=bcol(b2i, coh), scale=is2x[:])
            nc.vector.tensor_tensor(h_out[coh][:], tmp[:],
                                    h_in[coh][:].bitcast(F32), ALU.add)
        emit_vq(cx, h_out, cbXA, cbXB, e2nX, BPC * 64, 64, C1, qd_ps, c_acc_arg,
                qd_close=(c_acc_arg is not None))
        return h_out

    h2c = emit_block23(rh1, h1c, D["w1t_b2"], D["w2t_b2"], D["cb2"], 6, 7,
                       "b2", None)
    rh2 = padded_plane("rh2", 10, BPC)
    for coh in range(2):
        ov = rh2[coh][:].rearrange("p (i r c) -> p i r c", i=BPC, r=10, c=10)
        nc.scalar.activation(
            ov[:, :, 1:9, 1:9],
            h2c[coh][:].bitcast(F32).rearrange("p (i r c) -> p i r c",
                                               i=BPC, r=8, c=8),
            ACTF.Relu, bias=0.0, scale=1.0)
    h3c = emit_block23(rh2, h2c, D["w1t_b3"], D["w2t_b3"], D["cb3"], 8, 9,
                       "b3", c_acc)
    if parts < 4:
        q_sb = p["fix"].tile([1, BPC], F32, tag="q_sb")
        nc.scalar.copy(q_sb[:], qd_ps)
        nc.sync.dma_start(out=o_quant[:, :], in_=q_sb[:])
        nc.sync.dma_start(out=o_counts[:, :], in_=c_acc[:])
        if o_dbg is not None:
            dl = ((h0c[0], 2048, 0), (h0c[1], 2048, 2048),
                  (h1c[0], 512, 4096), (h1c[1], 512, 4608),
                  (h2c[0], 512, 5120), (h2c[1], 512, 5632),
                  (h3c[0], 512, 6144), (h3c[1], 512, 6656))
            for t, w, off in dl:
                nc.sync.dma_start(out=o_dbg[:, off:off + w], in_=t[:].bitcast(F32))
        return

    # ---------------- head ----------------
    hf = [p["fix"].tile([P, BPC], F32, tag=f"hf_{c}", name=f"hf_{c}") for c in range(2)]
    for coh in range(2):
        for i in range(BPC):
            dmp = p["dump"].tile([P, 64], F32, tag="dump")
            nc.scalar.activation(dmp[:],
                                 h3c[coh][:, i * 64:(i + 1) * 64].bitcast(F32),
                                 ACTF.Relu, bias=0.0, scale=1.0,
                                 accum_out=hf[coh][:, i:i + 1])

    linT = p["fix"].tile([P, 2], F32, tag="linT")
    nc.sync.dma_start(out=linT[:, 0:1], in_=D["linT"][0:128, :])
    nc.sync.dma_start(out=linT[:, 1:2], in_=D["linT"][128:256, :])
    isl = emit_sigma(cx, linT, 256, 1, "lin")
    embT = p["fix"].tile([P, 200], F32, tag="embT")
    nc.sync.dma_start(out=embT[:, 0:100], in_=D["embT"][0:128, :])
    nc.sync.dma_start(out=embT[:, 100:200], in_=D["embT"][128:256, :])
    ise = emit_sigma(cx, embT, 256, 100, "emb")

    y_sb = p["fix"].tile([1, BPC], F32, tag="y_sb")
    nc.sync.dma_start(out=y_sb[:], in_=D["y8"][:, :])
    iot = p["fix"].tile([100, 1], F32, tag="iot")
    nc.sync.dma_start(out=iot[:], in_=D["iota100"][:, :])
    ybc_ps = cx.ps_misc[0:100, 0:BPC]
    nc.tensor.matmul(ybc_ps, cx.ones_row[0:1, 0:100].bitcast(F32), y_sb[:],
                     start=True, stop=True)
    oh = p["fix"].tile([100, BPC], F32, tag="oh")
    nc.vector.tensor_scalar(oh[:], ybc_ps, iot[:], None, ALU.is_equal)
    emb_sb = p["fix"].tile([100, 256], F32, tag="emb_sb")
    nc.sync.dma_start(out=emb_sb[:], in_=D["emb_nat"][:, :])

    Fq = []
    for coh in range(2):
        es_ps = cx.ps_misc[0:P, 0:BPC]
        nc.tensor.matmul(es_ps, emb_sb[:, 128 * coh:128 * coh + 128], oh[:],
                         start=True, stop=True)
        linsc = p["fix"].tile([P, 1], F32, tag=cx.tag("linsc"))
        nc.vector.tensor_scalar(linsc[:], linT[:, coh:coh + 1],
                                isl[:], None, ALU.mult)
        A_sb = p["fix"].tile([P, BPC], F32, tag=cx.tag("A_sb"))
        nc.scalar.activation(A_sb[:], es_ps, ACTF.Identity,
                             bias=linsc[:], scale=ise[:])
        f = p["fix"].tile([P, BPC], F32, tag=cx.tag("Fq"))
        nc.vector.tensor_tensor(f[:], hf[coh][:], A_sb[:], ALU.mult)
        Fq.append(f)
    out_ps = cx.ps_misc[0:1, 0:BPC]
    for coh in range(2):
        nc.tensor.matmul(out_ps, cx.ones_col.bitcast(F32), Fq[coh][:],
                         start=(coh == 0), stop=(coh == 1))
    lb = p["fix"].tile([1, 1], F32, tag="lb")
    nc.sync.dma_start(out=lb[:], in_=D["lin_b"][:, :])
    o_sb = p["fix"].tile([1, BPC], F32, tag="o_sb")
    nc.scalar.activation(o_sb[:], out_ps, ACTF.Identity, bias=lb[:], scale=1.0)
    nc.sync.dma_start(out=o_out[:, :], in_=o_sb[:])

    q_sb = p["fix"].tile([1, BPC], F32, tag="q_sb")
    nc.scalar.copy(q_sb[:], qd_ps[:])
    nc.sync.dma_start(out=o_quant[:, :], in_=q_sb[:])
    nc.sync.dma_start(out=o_counts[:, :], in_=c_acc[:])

    if o_dbg is not None:
        off = 0
        for t, w in ((h0c[0], 2048), (h0c[1], 2048), (h1c[0], 512), (h1c[1], 512),
                     (h2c[0], 512), (h2c[1], 512), (h3c[0], 512), (h3c[1], 512),
                     (hf[0], 8), (hf[1], 8)):
            nc.sync.dma_start(out=o_dbg[:, off:off + w], in_=t[:].bitcast(F32))
            off += w


# ----------------------------------------------------------------- host side

_NC_CACHE = {}


def _get_nc(dev=0, parts=4):
    key = (dev, parts)
    if key not in _NC_CACHE:
        _NC_CACHE[key] = build_module(dev, parts)
    return _NC_CACHE[key]


def _wt3x3(w):
    co, ci = w.shape[0], w.shape[1]
    return _r32r(np.ascontiguousarray(
        np.asarray(w).reshape(co, ci, 3, 3).transpose(2, 3, 1, 0).reshape(
            9 * ci, co)))


def prep_core_inputs(inputs, core):
    d = {}
    s = slice(core * BPC, (core + 1) * BPC)
    d["x8"] = _r32r(inputs["x"][s])
    w1b0 = np.asarray(inputs["b0_w1"]).reshape(256, 3, 3, 3).transpose(2, 3, 1, 0)
    w1b0 = w1b0.reshape(9, 3, 256)[TAP_ORDER].reshape(27, 256)
    d["w1t_b0"] = _r32r(np.ascontiguousarray(w1b0))
    d["w2t_b0"] = _wt3x3(inputs["b0_w2"])
    d["w1t_b1"] = _wt3x3(inputs["b1_w1"])
    d["w2t_b1"] = _wt3x3(inputs["b1_w2"])
    d["w1t_b2"] = _wt3x3(inputs["b2_w1"])
    d["w2t_b2"] = _wt3x3(inputs["b2_w2"])
    d["w1t_b3"] = _wt3x3(inputs["b3_w1"])
    d["w2t_b3"] = _wt3x3(inputs["b3_w2"])
    d["wsct_b0"] = _r32r(np.asarray(inputs["b0_wsc"]).reshape(256, 3).T)
    d["wsct_b1"] = _r32r(np.asarray(inputs["b1_wsc"]).reshape(256, 256).T)
    biases = np.stack([
        inputs["b0_b1"], inputs["b0_b2"], inputs["b0_bsc"],
        inputs["b1_b1"], inputs["b1_b2"], inputs["b1_bsc"],
        inputs["b2_b1"], inputs["b2_b2"],
        inputs["b3_b1"], inputs["b3_b2"]], axis=1).astype(np.float32)
    d["biases"] = np.ascontiguousarray(biases)
    for i in range(4):
        d[f"cb{i}"] = _r32r(inputs[f"cb{i}"])
    d["linT"] = _r32r(np.asarray(inputs["lin_w"]).reshape(1, 256).T)
    d["lin_b"] = np.asarray(inputs["lin_b"], np.float32).reshape(1, 1)
    d["embT"] = _r32r(np.asarray(inputs["emb_w"]).T)
    d["emb_nat"] = _r32r(inputs["emb_w"])
    d["y8"] = _r32r(np.asarray(inputs["y"][s], np.float32).reshape(1, BPC))
    d["iota100"] = np.arange(100, dtype=np.float32).reshape(100, 1)
    cc = np.zeros((P, 8), np.float32)
    cc[:, 0] = 1.0
    cc[:, 1] = C0
    cc[:, 2] = -2.0 * C0
    cc[:, 3] = C1
    cc[:, 4] = -2.0 * C1
    cc[0:64, 6] = -2.0 * C1
    cc[64:128, 7] = -2.0 * C1
    d["cc"] = _r32r(cc)
    d["ones_row_d"] = np.ones((1, P), np.float32)
    d["zpad"] = np.zeros((P, 2592), np.float32)
    return d


def run_cores(inputs, dev=0, **kw):
    nc = _get_nc(dev)
    in_maps = [prep_core_inputs(inputs, c) for c in range(NCORES)]
    return run_bass_kernel_spmd(nc, in_maps, core_ids=list(range(NCORES)), **kw)


def assemble(results):
    outs = np.concatenate([r["o_out"][0] for r in results]).reshape(64, 1)
    quant = np.concatenate([r["o_quant"][0] for r in results]).reshape(64, 1)
    counts = np.sum([r["o_counts"][0] for r in results], axis=0).astype(np.float32)
    probs = counts / np.float32(64 * 8 * 8)
    ppl = np.exp(-np.sum(probs * np.log(probs + np.float32(1e-10)),
                         dtype=np.float32)).astype(np.float32)
    return (outs.astype(np.float32), quant.astype(np.float32), ppl)


def kernel(**inputs):
    inputs = {k: np.asarray(v) for k, v in inputs.items()}
    res = run_cores(inputs)
    return assemble(res.results)


# revision 26
# speedup vs baseline: 1.3192x; 1.0062x over previous
"""Trainium2 Bass kernel for nn_Discriminator_61332132987171 (vq_codebook).

Data-parallel over batch: 8 images per NeuronCore across 8 cores.
All matmuls in float32r (fp32 with 11-bit mantissa, full PE rate at N>=256).

Per-core pipeline:
  block0: im2col conv 3->256 (K=27), conv 256->256 (9-tap accumulation over a
          zero-padded SBUF plane), avgpool2, 1x1-conv shortcut on pooled x
  VQ0..VQ3: T[pix,code] = x.e - |e|^2/2 via matmuls (codes on free dim),
          max-reduce over codes, per-image sums into one PSUM accumulator
  block1: preact block at 16x16 with downsample + 1x1 shortcut
  block2/3: preact blocks at 8x8, image-batched matmuls (3D moving APs)
  head: hf = sum relu(h3) via ACT accum_out; out = hf . (lin/sl + emb[y]/se) + b
  spectral-norm 1/sigma for every weight computed on device and folded into
  the PSUM-eviction activation scale.

Host side only shards/transposes/rounds inputs and reduces per-core histogram
counts into the final perplexity scalar.
"""
import sys

for _p in ("/opt/trn_rl_repo", "/opt/pypackages"):
    if _p not in sys.path:
        sys.path.append(_p)

import numpy as np
import concourse.bass as bass  # noqa: F401
import concourse.mybir as mybir
import concourse.tile as tile
from concourse import bacc
from concourse.bass_utils import run_bass_kernel_spmd

F32 = mybir.dt.float32
F32R = mybir.dt.float32r
AX = mybir.AxisListType
ALU = mybir.AluOpType
ACTF = mybir.ActivationFunctionType

P = 128
NCORES = 8
BPC = 8  # images per core

C0 = 0.5 / (256.0 * 16 * 16)   # quant-loss scale, block0
C1 = 0.5 / (256.0 * 8 * 8)     # blocks 1-3
TAP_ORDER = [4, 0, 1, 2, 3, 5, 6, 7, 8]  # center tap first (partitions 0..2)


def _r32r(x):
    """Round fp32 -> fp32r (11-bit mantissa, RTNE) on host."""
    u = np.ascontiguousarray(x, dtype=np.float32).view(np.uint32)
    u2 = u + 0x7FF + ((u >> 12) & 1)
    return (u2 & 0xFFFFF000).astype(np.uint32).view(np.float32)


class Ctx:
    def __init__(self, nc, tc, pools, inp):
        self.nc = nc
        self.tc = tc
        self.p = pools
        self.inp = inp
        self.uid = 0
        self.qd_first = True

    def tag(self, base):
        self.uid += 1
        return f"{base}{self.uid}"


def emit_bcast11(cx, src11_f32_ap, tagbase):
    """[1,1] f32 AP -> [128,1] f32 SBUF column (K=1 ones matmul broadcast)."""
    nc, p = cx.nc, cx.p
    s_r = p["fix"].tile([1, 1], F32, tag=cx.tag("bc_r"))
    nc.vector.tensor_copy(s_r[:], src11_f32_ap)
    bc_ps = cx.ps_misc[0:P, 0:1]
    nc.tensor.matmul(bc_ps, cx.ones_row[0:1, 0:P].bitcast(F32), s_r[:],
                     start=True, stop=True)
    col = p["fix"].tile([P, 1], F32, tag=cx.tag(tagbase))
    nc.vector.tensor_copy(col[:], bc_ps)
    return col


def emit_sigma(cx, wt_sb, K, M, name):
    """1/sigma (spectral norm, 1 power iter) for WT layout [K, M-per-ktile]:
    k-tile j lives at wt_sb[:, j*M:(j+1)*M].  Returns [128,1] f32 column."""
    nc, p = cx.nc, cx.p
    kt = (K + P - 1) // P
    mh = (M + P - 1) // P
    kp = min(P, K)

    vtmp = p["fix"].tile([kp, kt], F32, tag=cx.tag("sg_vt"))
    for j in range(kt):
        pt = min(P, K - P * j)
        nc.vector.tensor_reduce(vtmp[0:pt, j:j + 1], wt_sb[0:pt, j * M:(j + 1) * M],
                                axis=AX.X, op=ALU.add)
    vcol = p["fix"].tile([kp, kt], F32, tag=cx.tag("sg_vc"))
    nc.vector.tensor_scalar(vcol[:], vtmp[:], 1.0 / float(np.sqrt(M)), None, ALU.mult)

    s_ps = cx.ps_misc[0:1, 0:1]
    for j in range(kt):
        pt = min(P, K - P * j)
        nc.tensor.matmul(s_ps, vcol[0:pt, j:j + 1], vcol[0:pt, j:j + 1],
                         start=(j == 0), stop=(j == kt - 1))
    s_sb = p["fix"].tile([1, 1], F32, tag=cx.tag("sg_s"))
    nc.vector.tensor_copy(s_sb[:], s_ps)
    a_sb = p["fix"].tile([1, 1], F32, tag=cx.tag("sg_a"))
    nc.scalar.sqrt(a_sb[:], s_sb[:])
    d_sb = p["fix"].tile([1, 1], F32, tag=cx.tag("sg_d"))
    nc.vector.tensor_scalar(d_sb[:], a_sb[:], 1e-8, None, ALU.add)
    r_sb = p["fix"].tile([1, 1], F32, tag=cx.tag("sg_r"))
    nc.vector.reciprocal(r_sb[:], d_sb[:])
    rcol = emit_bcast11(cx, r_sb[:], "sg_rc")

    vhat = p["fix"].tile([kp, kt], F32, tag=cx.tag("sg_vh"))
    nc.vector.tensor_scalar(vhat[:], vcol[:], rcol[0:kp, :], None, ALU.mult)

    wv_ps = cx.ps_misc[0:P, 0:mh]
    for h in range(mh):
        mw = min(P, M - P * h)
        for j in range(kt):
            pt = min(P, K - P * j)
            nc.tensor.matmul(
                cx.ps_misc[0:mw, h:h + 1],
                wt_sb[0:pt, j * M + h * P: j * M + h * P + mw].bitcast(F32),
                vhat[0:pt, j:j + 1],
                start=(h == 0 and j == 0), stop=(h == mh - 1 and j == kt - 1))
    wv_sb = p["fix"].tile([P, mh], F32, tag=cx.tag("sg_wv"))
    for h in range(mh):
        mw = min(P, M - P * h)
        nc.vector.tensor_copy(wv_sb[0:mw, h:h + 1], cx.ps_misc[0:mw, h:h + 1])

    s2_ps = cx.ps_misc[0:1, 0:1]
    for h in range(mh):
        mw = min(P, M - P * h)
        nc.tensor.matmul(s2_ps, wv_sb[0:mw, h:h + 1], wv_sb[0:mw, h:h + 1],
                         start=(h == 0), stop=(h == mh - 1))
    s2_sb = p["fix"].tile([1, 1], F32, tag=cx.tag("sg_s2"))
    nc.vector.tensor_copy(s2_sb[:], s2_ps)
    a2 = p["fix"].tile([1, 1], F32, tag=cx.tag("sg_a2"))
    nc.scalar.sqrt(a2[:], s2_sb[:])
    d2 = p["fix"].tile([1, 1], F32, tag=cx.tag("sg_d2"))
    nc.vector.tensor_scalar(d2[:], a2[:], 1e-8, None, ALU.add)
    r2 = p["fix"].tile([1, 1], F32, tag=cx.tag("sg_r2"))
    nc.vector.reciprocal(r2[:], s2_sb[:])
    inv11 = p["fix"].tile([1, 1], F32, tag=cx.tag("sg_i"))
    nc.vector.tensor_mul(inv11[:], d2[:], r2[:])
    return emit_bcast11(cx, inv11[:], f"invs_{name}")


def emit_load_weight(cx, dram, K, M, name):
    """DMA WT [K, M] -> SBUF k-tiles along free dim; compute 1/sigma."""
    nc, p = cx.nc, cx.p
    kt = (K + P - 1) // P
    if K > 256:
        wt_sb = p["wbig"].tile([P, kt * M], F32R, tag="wbig")
    else:
        wt_sb = p["fix"].tile([min(P, K), kt * M], F32R, tag=cx.tag(f"w_{name}"))
    for j in range(kt):
        pt = min(P, K - P * j)
        nc.sync.dma_start(out=wt_sb[0:pt, j * M:(j + 1) * M],
                          in_=dram[j * P: j * P + pt, :])
    invs = emit_sigma(cx, wt_sb, K, M, name)
    return wt_sb, invs


def emit_load_cb(cx, dram):
    """Codebook [256,1024] -> (cbA, cbB, e2neg) with e2neg = -|e_k|^2/2."""
    nc, p = cx.nc, cx.p
    cbA = p["cbA"].tile([P, 1024], F32R, tag="cbA")
    cbB = p["cbB"].tile([P, 1024], F32R, tag="cbB")
    nc.sync.dma_start(out=cbA[:], in_=dram[0:128, :])
    nc.sync.dma_start(out=cbB[:], in_=dram[128:256, :])
    e2neg = p["e2"].tile([1, 1024], F32R, tag="e2")
    for kh in range(2):
        ksl = slice(512 * kh, 512 * kh + 512)
        sq = p["tsb"].tile([P, 1024], F32R, tag="tsb")
        nc.vector.tensor_tensor(sq[:, 0:512], cbA[:, ksl].bitcast(F32),
                                cbA[:, ksl].bitcast(F32), ALU.mult)
        nc.vector.tensor_tensor(sq[:, 512:1024], cbB[:, ksl].bitcast(F32),
                                cbB[:, ksl].bitcast(F32), ALU.mult)
        e2_ps = cx.ps_misc[0:1, 0:512]
        nc.tensor.matmul(e2_ps, cx.ones_col, sq[:, 0:512], start=True, stop=False)
        nc.tensor.matmul(e2_ps, cx.ones_col, sq[:, 512:1024], start=False, stop=True)
        nc.scalar.activation(e2neg[0:1, ksl], e2_ps, ACTF.Copy,
                             bias=0.0, scale=-0.5)
    return cbA, cbB, e2neg


def emit_vq(cx, h_cmp, cbA, cbB, e2neg, n_pix, pix_per_img, c_scale, qd_ps,
            c_acc, qd_close=False, vq_mode="full", imgs=None):
    """VQ pass over compact activations h_cmp = 2 x [128, n_pix] (f32r).

    T[pix, code] = x.e - |e|^2/2;  min_dist = |x|^2 - 2*max_code(T).
    Accumulates c*sum(h^2) - 2c*sum(Tmax) per image into qd_ps[0, img].
    If c_acc is not None, adds code histogram counts into it ([1,1024] f32).
    """
    nc, p = cx.nc, cx.p
    ntile = n_pix // P
    n_img = n_pix // pix_per_img
    img_per_tile = max(1, P // pix_per_img)
    tile_per_img = max(1, pix_per_img // P)
    cpos = cx.cpos0 if c_scale == C0 else cx.cpos1
    cneg = cx.cneg0 if c_scale == C0 else cx.cneg1
    if imgs is None:
        imgs = range(n_img)
    tiles = sorted({(i * tile_per_img + k) // img_per_tile
                    for i in imgs for k in range(tile_per_img)})

    for i in imgs:
        if vq_mode == "T":
            break
        for coh in range(2):
            hsl = h_cmp[coh][:, i * pix_per_img:(i + 1) * pix_per_img].bitcast(F32)
            dump = p["dump"].tile([P, pix_per_img], F32, tag="dump")
            sqa = p["cols"].tile([P, 1], F32, tag="cols")
            nc.vector.tensor_tensor(dump[:], hsl, hsl, ALU.mult)
            nc.vector.tensor_reduce(sqa[:], dump[:], axis=AX.X, op=ALU.add)
            if vq_mode == "ttr":
                nc.sync.dma_start(out=cx.dbg_T[:, 2 * i + coh: 2 * i + coh + 1],
                                  in_=sqa[:])
                continue
            nc.tensor.matmul(qd_ps[0:1, i:i + 1], sqa[:], cpos.bitcast(F32),
                             start=cx.qd_first,
                             stop=(vq_mode == "sq" and i == n_img - 1 and coh == 1))
            cx.qd_first = False

    if vq_mode in ("sq", "ttr"):
        return
    for t in tiles:
        pix0 = t * P
        T_sb = p["tsb"].tile([P, 1024], F32, tag="tsb")
        for kh in range(2):
            ksl = slice(512 * kh, 512 * kh + 512)
            T_ps = p["vq"].tile([P, 512], F32, tag="vq")
            nc.tensor.matmul(T_ps[:], h_cmp[0][:, pix0:pix0 + P], cbA[:, ksl],
                             start=True, stop=False)
            nc.tensor.matmul(T_ps[:], h_cmp[1][:, pix0:pix0 + P], cbB[:, ksl],
                             start=False, stop=False)
            nc.tensor.matmul(T_ps[:], cx.ones_row[0:1, 0:P], e2neg[0:1, ksl],
                             start=False, stop=True)
            nc.scalar.copy(T_sb[:, ksl], T_ps[:])

        tmx = p["cols"].tile([P, 1], F32, tag="cols")
        nc.vector.tensor_reduce(tmx[:], T_sb[:], axis=AX.X, op=ALU.max)
        if vq_mode == "T":
            nc.sync.dma_start(out=cx.dbg_T[:, t:t + 1], in_=tmx[:])
            continue
        last = qd_close and t == ntile - 1
        if img_per_tile <= 1:
            i = t // tile_per_img
            nc.tensor.matmul(qd_ps[0:1, i:i + 1], tmx[:], cneg.bitcast(F32),
                             start=False, stop=last)
        else:
            assert img_per_tile == 2
            i0 = t * 2
            nc.tensor.matmul(qd_ps[0:1, i0:i0 + 2], tmx[:],
                             cx.ind2.bitcast(F32), start=False, stop=last)

        if c_acc is not None:
            eq = p["tsb"].tile([P, 1024], F32R, tag="tsb")
            nc.vector.tensor_scalar(eq[:], T_sb[:], tmx[:], None, ALU.is_equal)
            for kh in range(2):
                cnt_ps = p["sc"].tile([1, 512], F32, tag="conv")
                nc.tensor.matmul(cnt_ps[:], cx.ones_col,
                                 eq[:, 512 * kh:512 * kh + 512],
                                 start=True, stop=True)
                nc.vector.tensor_tensor(c_acc[0:1, 512 * kh:512 * kh + 512],
                                        c_acc[0:1, 512 * kh:512 * kh + 512],
                                        cnt_ps[:], ALU.add)


def emit_pool2(cx, src_ap, rows, cols):
    """2x2 sum-pool of SBUF tensor [128, rows*cols] -> dump tile [128, rows*cols/4]."""
    nc, p = cx.nc, cx.p
    hc = cols // 2
    hr = rows // 2
    t1 = p["dump"].tile([P, rows * hc], F32, tag="dump")
    v = src_ap.rearrange("p (r c two) -> p r c two", r=rows, c=hc, two=2)
    nc.vector.tensor_tensor(t1[:], v[:, :, :, 0], v[:, :, :, 1], ALU.add)
    t1v = t1[:].rearrange("p (r two c) -> p r two c", r=hr, two=2, c=hc)
    t2 = p["dump"].tile([P, hr * hc], F32, tag="dump")
    nc.vector.tensor_tensor(t2[:], t1v[:, :, 0, :], t1v[:, :, 1, :], ALU.add)
    return t2


def emit_pool_fuse(cx, y_ps_ap, sc_sb_ap, h_out_ap, rows, cols, scale_col,
                   bsum_col):
    """h_out = poolsum2x2(y_ps * scale_col) + bsum_col + sc_sb."""
    nc, p = cx.nc, cx.p
    y_sb = p["dump"].tile([P, rows * cols], F32, tag="dump")
    nc.scalar.activation(y_sb[:], y_ps_ap, ACTF.Identity, bias=0.0,
                         scale=scale_col[:])
    t2 = emit_pool2(cx, y_sb[:], rows, cols)
    nc.vector.scalar_tensor_tensor(h_out_ap, t2[:], bsum_col, sc_sb_ap,
                                   op0=ALU.add, op1=ALU.add)


DBG_WIDTH = 4096 + 1024 + 1024 + 1024 + 16


def build_module(dev=0, parts=4):
    nc = bacc.Bacc("TRN2", target_bir_lowering=False, debug=False)
    D = {}

    def din(name, shape, dt=F32R):
        D[name] = nc.dram_tensor(name, shape, dt, kind="ExternalInput")

    din("x8", [BPC, 3, 32, 32])
    din("w1t_b0", [27, 256])
    for nm in ("w2t_b0", "w1t_b1", "w2t_b1", "w1t_b2", "w2t_b2", "w1t_b3",
               "w2t_b3"):
        din(nm, [2304, 256])
    din("wsct_b0", [3, 256])
    din("wsct_b1", [256, 256])
    din("biases", [256, 10], F32)
    for i in range(4):
        din(f"cb{i}", [256, 1024])
    din("linT", [256, 1], F32)
    din("lin_b", [1, 1], F32)
    din("embT", [256, 100], F32)
    din("emb_nat", [100, 256], F32)
    din("y8", [1, BPC], F32)
    din("iota100", [100, 1], F32)
    din("cc", [P, 8])
    din("ones_row_d", [1, P])
    din("zpad", [P, 2592])

    o_out = nc.dram_tensor("o_out", [1, BPC], F32, kind="ExternalOutput")
    o_quant = nc.dram_tensor("o_quant", [1, BPC], F32, kind="ExternalOutput")
    o_counts = nc.dram_tensor("o_counts", [1, 1024], F32, kind="ExternalOutput")
    o_dbg = None
    if dev:
        o_dbg = nc.dram_tensor("o_dbg", [P, DBG_WIDTH], F32, kind="ExternalOutput")

    with tile.TileContext(nc) as tc:
        with (
            tc.tile_pool(name="fix", bufs=1) as fix,
            tc.tile_pool(name="wbig", bufs=2) as wbig,
            tc.tile_pool(name="cbA", bufs=2) as cbA_p,
            tc.tile_pool(name="cbB", bufs=2) as cbB_p,
            tc.tile_pool(name="e2", bufs=2) as e2_p,
            tc.tile_pool(name="tsb", bufs=2) as tsb,
            tc.tile_pool(name="dump", bufs=5) as dump,
            tc.tile_pool(name="cols", bufs=8) as cols,
            tc.tile_pool(name="conv", bufs=4, space="PSUM") as conv_ps,
            tc.tile_pool(name="vq", bufs=2, space="PSUM") as vq_psp,
            tc.tile_pool(name="qd", bufs=1, space="PSUM") as qd_p,
            tc.tile_pool(name="misc", bufs=1, space="PSUM") as misc,
        ):
            pools = dict(fix=fix, wbig=wbig, cbA=cbA_p, cbB=cbB_p, e2=e2_p,
                         tsb=tsb, dump=dump, cols=cols, conv=conv_ps, sc=conv_ps,
                         vq=vq_psp, qd=qd_p, misc=misc)
            cx = Ctx(nc, tc, pools, D)
            emit_all(cx, D, o_out, o_quant, o_counts, o_dbg, parts)

    nc.compile()
    return nc


def emit_all(cx, D, o_out, o_quant, o_counts, o_dbg, parts=4):
    nc, p = cx.nc, cx.p

    # ---- consts / biases
    cc = p["fix"].tile([P, 8], F32R, tag="cc")
    nc.sync.dma_start(out=cc[:], in_=D["cc"][:, :])
    cx.ones_col = cc[:, 0:1]
    cx.cpos0 = cc[:, 1:2]
    cx.cneg0 = cc[:, 2:3]
    cx.cpos1 = cc[:, 3:4]
    cx.cneg1 = cc[:, 4:5]
    cx.ind2 = cc[:, 6:8]
    ones_row = p["fix"].tile([1, P], F32R, tag="ones_row")
    nc.sync.dma_start(out=ones_row[:], in_=D["ones_row_d"][:, :])
    cx.ones_row = ones_row

    bias = p["fix"].tile([P, 20], F32, tag="bias")
    nc.sync.dma_start(out=bias[:, 0:10], in_=D["biases"][0:128, :])
    nc.sync.dma_start(out=bias[:, 10:20], in_=D["biases"][128:256, :])

    def bcol(idx, coh):
        return bias[:, 10 * coh + idx: 10 * coh + idx + 1]

    bsum = p["fix"].tile([P, 4], F32, tag="bsum")
    nc.vector.tensor_tensor(bsum[:, 0:1], bcol(2, 0), bcol(1, 0), ALU.add)
    nc.vector.tensor_tensor(bsum[:, 1:2], bcol(2, 1), bcol(1, 1), ALU.add)
    nc.vector.tensor_tensor(bsum[:, 2:3], bcol(5, 0), bcol(4, 0), ALU.add)
    nc.vector.tensor_tensor(bsum[:, 3:4], bcol(5, 1), bcol(4, 1), ALU.add)

    qd_full = p["qd"].tile([1, 512], F32, tag="qd")
    qd_ps = qd_full[0:1, 0:BPC]
    cx.ps_misc = p["misc"].tile([P, 512], F32, tag="misc")
    c_acc = p["fix"].tile([1, 1024], F32, tag="c_acc")
    nc.vector.memset(c_acc[:], 0.0)

    # ---- block0 weights
    w1, is1 = emit_load_weight(cx, D["w1t_b0"], 27, 256, "b0w1")
    w2, is2 = emit_load_weight(cx, D["w2t_b0"], 2304, 256, "b0w2")
    wsc, issc = emit_load_weight(cx, D["wsct_b0"], 3, 256, "b0wsc")
    cb0A, cb0B, e2n0 = emit_load_cb(cx, D["cb0"])

    def scale4(col, nm):
        out = p["fix"].tile([P, 1], F32, tag=cx.tag(nm))
        nc.vector.tensor_scalar(out[:], col[:], 0.25, None, ALU.mult)
        return out

    is2_4 = scale4(is2, "is2_4")
    issc_4 = scale4(issc, "issc_4")

    # im2col buffers, zeroed once (tap-valid regions identical per image)
    X0 = []
    for b in range(2):
        t = p["fix"].tile([27, 1024], F32R, tag=f"X0_{b}")
        nc.sync.dma_start(out=t[:], in_=D["zpad"][0:27, 0:1024])
        X0.append(t)

    def zero_borders(t, side, nimg):
        n = nimg * side * side
        nc.sync.dma_start(out=t[:, 0:n], in_=D["zpad"][:, 0:n])

    def padded_plane(tagbase, side, nimg):
        pair = []
        for coh in range(2):
            t = p["fix"].tile([P, nimg * side * side], F32R,
                              tag=f"{tagbase}_{coh}")
            zero_borders(t, side, nimg)
            pair.append(t)
        return pair

    y1p = []
    for b in range(2):
        pair = []
        for coh in range(2):
            t = p["fix"].tile([P, 34 * 34], F32R, tag=f"y1p_{b}_{coh}")
            zero_borders(t, 34, 1)
            pair.append(t)
        y1p.append(pair)

    rh0 = padded_plane("rh0", 18, BPC)
    h0c = [p["fix"].tile([P, BPC * 256], F32R, tag=f"h0c_{c}", name=f"h0c_{c}") for c in range(2)]

    # ---------------- block 0 ----------------
    if parts < 0.2:
        if o_dbg is not None:
            nc.sync.dma_start(out=o_dbg[:, 0:1], in_=is1[:])
            nc.sync.dma_start(out=o_dbg[:, 1:2], in_=is2[:])
            nc.sync.dma_start(out=o_dbg[:, 2:3], in_=issc[:])
            nc.sync.dma_start(out=o_dbg[0:1, 16:1040], in_=e2n0[:].bitcast(F32))
        return
    dma_eng = [nc.sync, nc.sync, nc.sync]
    for i in range(BPC):
        xb = X0[i % 2]
        dstv = xb[:].rearrange("p (r c) -> p r c", r=32, c=32)
        for pos, t9 in enumerate(TAP_ORDER):
            dy, dx = t9 // 3, t9 % 3
            hlo, hhi = max(0, 1 - dy), min(32, 33 - dy)
            wlo, whi = max(0, 1 - dx), min(32, 33 - dx)
            dma_eng[pos % 3].dma_start(
                out=dstv[3 * pos:3 * pos + 3, hlo:hhi, wlo:whi],
                in_=D["x8"][i, :, hlo + dy - 1:hhi + dy - 1,
                            wlo + dx - 1:whi + dx - 1])
        yb = y1p[i % 2]
        if parts < 0.4:
            if o_dbg is not None:
                nc.sync.dma_start(out=o_dbg[0:27, 1040 + i * 128:1040 + i * 128 + 128],
                                  in_=xb[:, 0:128].bitcast(F32))
            continue
        for s in range(2):
            for coh in range(2):
                ps = p["conv"].tile([P, 512], F32, tag="conv")
                nc.tensor.matmul(ps[:], w1[0:27, 128 * coh:128 * coh + 128],
                                 xb[:, 512 * s:512 * s + 512], start=True, stop=True)
                ov = yb[coh][:].rearrange("p (r c) -> p r c", r=34, c=34)
                nc.scalar.activation(ov[:, 1 + 16 * s:17 + 16 * s, 1:33], ps[:],
                                     ACTF.Relu, bias=bcol(0, coh), scale=is1[:])
        if parts < 0.6:
            if o_dbg is not None:
                nc.sync.dma_start(out=o_dbg[:, 1040 + i * 128:1040 + i * 128 + 128],
                                  in_=yb[0][:, 0:128].bitcast(F32))
            continue
        # pooled x (sum form) from im2col center tap (rows 12..14 = full image)
        xt1 = p["dump"].tile([3, 512], F32, tag="dump")
        xcv = xb[0:3, :].rearrange("p (r c two) -> p r c two", r=32, c=16, two=2)
        nc.vector.tensor_tensor(xt1[:], xcv[:, :, :, 0], xcv[:, :, :, 1], ALU.add)
        xp_i = p["dump"].tile([3, 256], F32R, tag="dump")
        xt1v = xt1[:].rearrange("p (r two c) -> p r two c", r=16, two=2, c=16)
        nc.vector.tensor_tensor(xp_i[:], xt1v[:, :, 0, :], xt1v[:, :, 1, :], ALU.add)
        sc_psT = p["sc"].tile([P, 512], F32, tag="conv")
        for coh in range(2):
            nc.tensor.matmul(sc_psT[:, 256 * coh:256 * coh + 256],
                             wsc[0:3, 128 * coh:128 * coh + 128],
                             xp_i[:], start=(coh == 0), stop=(coh == 1))
        for s in range(2):
            c2 = []
            for coh in range(2):
                ps = p["conv"].tile([P, 512], F32, tag="conv")
                first = True
                for t9 in range(9):
                    dy, dx = t9 // 3, t9 % 3
                    for cih in range(2):
                        src = yb[cih][:].rearrange("p (r c) -> p r c", r=34, c=34)
                        rhs = src[:, dy + 16 * s: dy + 16 * s + 16, dx:dx + 32]
                        nc.tensor.matmul(
                            ps[:],
                            w2[:, (2 * t9 + cih) * 256 + 128 * coh:
                                  (2 * t9 + cih) * 256 + 128 * coh + 128],
                            rhs, start=first, stop=(t9 == 8 and cih == 1))
                        first = False
                c2.append(ps)
            for coh in range(2):
                sc_sb = p["dump"].tile([P, 128], F32, tag="dump")
                scv = sc_psT[:, 256 * coh:256 * coh + 256].rearrange(
                    "p (r c) -> p r c", r=16, c=16)
                nc.scalar.activation(
                    sc_sb[:].rearrange("p (r c) -> p r c", r=8, c=16),
                    scv[:, 8 * s:8 * s + 8, :],
                    ACTF.Identity, bias=0.0, scale=issc_4[:])
                emit_pool_fuse(
                    cx, c2[coh][:], sc_sb[:],
                    h0c[coh][:, i * 256 + 128 * s: i * 256 + 128 * s + 128],
                    16, 32, is2_4, bsum[:, coh:coh + 1])
        for coh in range(2):
            ov = rh0[coh][:].rearrange("p (i r c) -> p i r c", i=BPC, r=18, c=18)
            nc.scalar.activation(
                ov[:, i, 1:17, 1:17],
                h0c[coh][:, i * 256:(i + 1) * 256].bitcast(F32).rearrange(
                    "p (r c) -> p r c", r=16, c=16),
                ACTF.Relu, bias=0.0, scale=1.0)
        if parts >= 1 and parts not in (0.8, 0.85, 0.9):
            emit_vq(cx, h0c, cb0A, cb0B, e2n0, BPC * 256, 256, C0, qd_ps, None,
                    qd_close=False, imgs=[i])

    if parts in (0.8, 0.85, 0.9):
        pass
    elif parts < 1:
        if o_dbg is not None and parts >= 0.6:
            for t, w, off in ((h0c[0], 2048, 0), (h0c[1], 2048, 2048)):
                nc.sync.dma_start(out=o_dbg[:, off:off + w], in_=t[:].bitcast(F32))
        return
    if parts in (0.8, 0.85, 0.9):
        cx.dbg_T = o_dbg
        emit_vq(cx, h0c, cb0A, cb0B, e2n0, BPC * 256, 256, C0, qd_ps, None,
                qd_close=False,
                vq_mode={0.8: "sq", 0.85: "ttr", 0.9: "T"}[parts])
        if parts == 0.8:
            q_sb = p["fix"].tile([1, BPC], F32, tag="q_sb")
            nc.scalar.copy(q_sb[:], qd_ps)
            nc.sync.dma_start(out=o_quant[:, :], in_=q_sb[:])
        return
    if parts >= 2:
        emit_rest(cx, D, o_out, o_quant, o_counts, o_dbg, parts,
                  bcol, bsum, qd_ps, c_acc, h0c, rh0, padded_plane, scale4,
                  zero_borders)
    else:
        q_sb = p["fix"].tile([1, BPC], F32, tag="q_sb")
        nc.scalar.copy(q_sb[:], qd_ps)
        nc.sync.dma_start(out=o_quant[:, :], in_=q_sb[:])
        if o_dbg is not None:
            for t, w, off in ((h0c[0], 2048, 0), (h0c[1], 2048, 2048)):
                nc.sync.dma_start(out=o_dbg[:, off:off + w], in_=t[:].bitcast(F32))


def emit_rest(cx, D, o_out, o_quant, o_counts, o_dbg, parts,
              bcol, bsum, qd_ps, c_acc, h0c, rh0, padded_plane, scale4,
              zero_borders):
    nc, p = cx.nc, cx.p
    # ---------------- block 1 ----------------
    w1b1, is1b1 = emit_load_weight(cx, D["w1t_b1"], 2304, 256, "b1w1")
    w2b1, is2b1 = emit_load_weight(cx, D["w2t_b1"], 2304, 256, "b1w2")
    wscb1, isscb1 = emit_load_weight(cx, D["wsct_b1"], 256, 256, "b1wsc")
    cb1A, cb1B, e2n1 = emit_load_cb(cx, D["cb1"])
    is2b1_4 = scale4(is2b1, "is2b1_4")
    isscb1_4 = scale4(isscb1, "isscb1_4")

    rt1 = []
    for b in range(2):
        pair = []
        for coh in range(2):
            t = p["fix"].tile([P, 2 * 18 * 18], F32R, tag=f"rt1_{b}_{coh}")
            zero_borders(t, 18, 2)
            pair.append(t)
        rt1.append(pair)

    rh1 = padded_plane("rh1", 10, BPC)
    h1c = [p["fix"].tile([P, BPC * 64], F32R, tag=f"h1c_{c}", name=f"h1c_{c}") for c in range(2)]

    for i0 in range(0, BPC, 2):
        rb = rt1[(i0 // 2) % 2]  # [coh] -> [128, 2*18*18] image-pair planes
        for coh in range(2):
            ps = p["conv"].tile([P, 512], F32, tag="conv")
            first = True
            for t9 in range(9):
                dy, dx = t9 // 3, t9 % 3
                for cih in range(2):
                    src = rh0[cih][:].rearrange("p (i r c) -> p i r c",
                                                i=BPC, r=18, c=18)
                    rhs = src[:, i0:i0 + 2, dy:dy + 16, dx:dx + 16]
                    nc.tensor.matmul(
                        ps[:],
                        w1b1[:, (2 * t9 + cih) * 256 + 128 * coh:
                                (2 * t9 + cih) * 256 + 128 * coh + 128],
                        rhs, start=first, stop=(t9 == 8 and cih == 1))
                    first = False
            ov = rb[coh][:].rearrange("p (i r c) -> p i r c", i=2, r=18, c=18)
            nc.scalar.activation(
                ov[:, :, 1:17, 1:17],
                ps[:].rearrange("p (i r c) -> p i r c", i=2, r=16, c=16),
                ACTF.Relu, bias=bcol(3, coh), scale=is1b1[:])
        sc_psT = p["sc"].tile([P, 512], F32, tag="conv")
        for coh in range(2):
            for cih in range(2):
                nc.tensor.matmul(
                    sc_psT[:, 256 * coh:256 * coh + 256],
                    wscb1[:, cih * 256 + 128 * coh: cih * 256 + 128 * coh + 128],
                    h0c[cih][:, i0 * 256:(i0 + 1) * 256],
                    start=(coh == 0 and cih == 0), stop=(coh == 1 and cih == 1))
        sc_psT2 = p["sc"].tile([P, 512], F32, tag="conv")
        for coh in range(2):
            for cih in range(2):
                nc.tensor.matmul(
                    sc_psT2[:, 256 * coh:256 * coh + 256],
                    wscb1[:, cih * 256 + 128 * coh: cih * 256 + 128 * coh + 128],
                    h0c[cih][:, (i0 + 1) * 256:(i0 + 2) * 256],
                    start=(coh == 0 and cih == 0), stop=(coh == 1 and cih == 1))
        for coh in range(2):
            ps = p["conv"].tile([P, 512], F32, tag="conv")
            first = True
            for t9 in range(9):
                dy, dx = t9 // 3, t9 % 3
                for cih in range(2):
                    src = rb[cih][:].rearrange("p (i r c) -> p i r c",
                                               i=2, r=18, c=18)
                    rhs = src[:, :, dy:dy + 16, dx:dx + 16]
                    nc.tensor.matmul(
                        ps[:],
                        w2b1[:, (2 * t9 + cih) * 256 + 128 * coh:
                                (2 * t9 + cih) * 256 + 128 * coh + 128],
                        rhs, start=first, stop=(t9 == 8 and cih == 1))
                    first = False
            for k, scp in ((0, sc_psT), (1, sc_psT2)):
                i = i0 + k
                sc_e = p["dump"].tile([P, 256], F32, tag="dump")
                nc.scalar.activation(sc_e[:], scp[:, 256 * coh:256 * coh + 256],
                                     ACTF.Identity, bias=0.0, scale=isscb1_4[:])
                sc_t2 = emit_pool2(cx, sc_e[:], 16, 16)
                emit_pool_fuse(cx, ps[:, k * 256:(k + 1) * 256], sc_t2[:],
                               h1c[coh][:, i * 64:(i + 1) * 64], 16, 16,
                               is2b1_4, bsum[:, 2 + coh:3 + coh])
        for coh in range(2):
            ov = rh1[coh][:].rearrange("p (i r c) -> p i r c", i=BPC, r=10, c=10)
            nc.scalar.activation(
                ov[:, i0:i0 + 2, 1:9, 1:9],
                h1c[coh][:, i0 * 64:(i0 + 2) * 64].bitcast(F32).rearrange(
                    "p (i r c) -> p i r c", i=2, r=8, c=8),
                ACTF.Relu, bias=0.0, scale=1.0)
        emit_vq(cx, h1c, cb1A, cb1B, e2n1, BPC * 64, 64, C1, qd_ps, None,
                qd_close=False, imgs=[i0, i0 + 1])
    if parts < 3:
        q_sb = p["fix"].tile([1, BPC], F32, tag="q_sb")
        nc.scalar.copy(q_sb[:], qd_ps)
        nc.sync.dma_start(out=o_quant[:, :], in_=q_sb[:])
        if o_dbg is not None:
            dl = ((h0c[0], 2048, 0), (h0c[1], 2048, 2048),
                  (h1c[0], 512, 4096), (h1c[1], 512, 4608))
            for t, w, off in dl:
                nc.sync.dma_start(out=o_dbg[:, off:off + w], in_=t[:].bitcast(F32))
        return

    # ---------------- blocks 2 & 3 ----------------
    def emit_block23(rh_in, h_in, w1d, w2d, cbd, b1i, b2i, nm, c_acc_arg):
        w1x, is1x = emit_load_weight(cx, w1d, 2304, 256, nm + "w1")
        w2x, is2x = emit_load_weight(cx, w2d, 2304, 256, nm + "w2")
        cbXA, cbXB, e2nX = emit_load_cb(cx, cbd)
        rt = padded_plane("rt_" + nm, 10, BPC)
        h_out = [p["fix"].tile([P, BPC * 64], F32R, tag=f"h_{nm}_{c}",
                               name=f"h_{nm}_{c}") for c in range(2)]
        for coh in range(2):
            ps = p["conv"].tile([P, 512], F32, tag="conv")
            first = True
            for t9 in range(9):
                dy, dx = t9 // 3, t9 % 3
                for cih in range(2):
                    src = rh_in[cih][:].rearrange("p (i r c) -> p i r c",
                                                  i=BPC, r=10, c=10)
                    rhs = src[:, :, dy:dy + 8, dx:dx + 8]
                    nc.tensor.matmul(
                        ps[:],
                        w1x[:, (2 * t9 + cih) * 256 + 128 * coh:
                                (2 * t9 + cih) * 256 + 128 * coh + 128],
                        rhs, start=first, stop=(t9 == 8 and cih == 1))
                    first = False
            ov = rt[coh][:].rearrange("p (i r c) -> p i r c", i=BPC, r=10, c=10)
            nc.scalar.activation(
                ov[:, :, 1:9, 1:9],
                ps[:].rearrange("p (i r c) -> p i r c", i=BPC, r=8, c=8),
                ACTF.Relu, bias=bcol(b1i, coh), scale=is1x[:])
        for coh in range(2):
            ps = p["conv"].tile([P, 512], F32, tag="conv")
            first = True
            for t9 in range(9):
                dy, dx = t9 // 3, t9 % 3
                for cih in range(2):
                    src = rt[cih][:].rearrange("p (i r c) -> p i r c",
                                               i=BPC, r=10, c=10)
                    rhs = src[:, :, dy:dy + 8, dx:dx + 8]
                    nc.tensor.matmul(
                        ps[:],
                        w2x[:, (2 * t9 + cih) * 256 + 128 * coh:
                                (2 * t9 + cih) * 256 + 128 * coh + 128],
                        rhs, start=first, stop=(t9 == 8 and cih == 1))
                    first = False
            tmp = p["dump"].tile([P, 512], F32, tag="dump")
            nc.scalar.activation(tmp[:], ps[:], ACTF.Identity,
                                 bias=bcol(b2i, coh), scale=is2x[:])
            nc.vector.tensor_tensor(h_out[coh][:], tmp[:],
                                    h_in[coh][:].bitcast(F32), ALU.add)
        emit_vq(cx, h_out, cbXA, cbXB, e2nX, BPC * 64, 64, C1, qd_ps, c_acc_arg,
                qd_close=(c_acc_arg is not None))
        return h_out

    h2c = emit_block23(rh1, h1c, D["w1t_b2"], D["w2t_b2"], D["cb2"], 6, 7,
                       "b2", None)
    rh2 = padded_plane("rh2", 10, BPC)
    for coh in range(2):
        ov = rh2[coh][:].rearrange("p (i r c) -> p i r c", i=BPC, r=10, c=10)
        nc.scalar.activation(
            ov[:, :, 1:9, 1:9],
            h2c[coh][:].bitcast(F32).rearrange("p (i r c) -> p i r c",
                                               i=BPC, r=8, c=8),
            ACTF.Relu, bias=0.0, scale=1.0)
    h3c = emit_block23(rh2, h2c, D["w1t_b3"], D["w2t_b3"], D["cb3"], 8, 9,
                       "b3", c_acc)
    if parts < 4:
        q_sb = p["fix"].tile([1, BPC], F32, tag="q_sb")
        nc.scalar.copy(q_sb[:], qd_ps)
        nc.sync.dma_start(out=o_quant[:, :], in_=q_sb[:])
        nc.sync.dma_start(out=o_counts[:, :], in_=c_acc[:])
        if o_dbg is not None:
            dl = ((h0c[0], 2048, 0), (h0c[1], 2048, 2048),
                  (h1c[0], 512, 4096), (h1c[1], 512, 4608),
                  (h2c[0], 512, 5120), (h2c[1], 512, 5632),
                  (h3c[0], 512, 6144), (h3c[1], 512, 6656))
            for t, w, off in dl:
                nc.sync.dma_start(out=o_dbg[:, off:off + w], in_=t[:].bitcast(F32))
        return

    # ---------------- head ----------------
    hf = [p["fix"].tile([P, BPC], F32, tag=f"hf_{c}", name=f"hf_{c}") for c in range(2)]
    for coh in range(2):
        for i in range(BPC):
            dmp = p["dump"].tile([P, 64], F32, tag="dump")
            nc.scalar.activation(dmp[:],
                                 h3c[coh][:, i * 64:(i + 1) * 64].bitcast(F32),
                                 ACTF.Relu, bias=0.0, scale=1.0,
                                 accum_out=hf[coh][:, i:i + 1])

    linT = p["fix"].tile([P, 2], F32, tag="linT")
    nc.sync.dma_start(out=linT[:, 0:1], in_=D["linT"][0:128, :])
    nc.sync.dma_start(out=linT[:, 1:2], in_=D["linT"][128:256, :])
    isl = emit_sigma(cx, linT, 256, 1, "lin")
    embT = p["fix"].tile([P, 200], F32, tag="embT")
    nc.sync.dma_start(out=embT[:, 0:100], in_=D["embT"][0:128, :])
    nc.sync.dma_start(out=embT[:, 100:200], in_=D["embT"][128:256, :])
    ise = emit_sigma(cx, embT, 256, 100, "emb")

    y_sb = p["fix"].tile([1, BPC], F32, tag="y_sb")
    nc.sync.dma_start(out=y_sb[:], in_=D["y8"][:, :])
    iot = p["fix"].tile([100, 1], F32, tag="iot")
    nc.sync.dma_start(out=iot[:], in_=D["iota100"][:, :])
    ybc_ps = cx.ps_misc[0:100, 0:BPC]
    nc.tensor.matmul(ybc_ps, cx.ones_row[0:1, 0:100].bitcast(F32), y_sb[:],
                     start=True, stop=True)
    oh = p["fix"].tile([100, BPC], F32, tag="oh")
    nc.vector.tensor_scalar(oh[:], ybc_ps, iot[:], None, ALU.is_equal)
    emb_sb = p["fix"].tile([100, 256], F32, tag="emb_sb")
    nc.sync.dma_start(out=emb_sb[:], in_=D["emb_nat"][:, :])

    Fq = []
    for coh in range(2):
        es_ps = cx.ps_misc[0:P, 0:BPC]
        nc.tensor.matmul(es_ps, emb_sb[:, 128 * coh:128 * coh + 128], oh[:],
                         start=True, stop=True)
        linsc = p["fix"].tile([P, 1], F32, tag=cx.tag("linsc"))
        nc.vector.tensor_scalar(linsc[:], linT[:, coh:coh + 1],
                                isl[:], None, ALU.mult)
        A_sb = p["fix"].tile([P, BPC], F32, tag=cx.tag("A_sb"))
        nc.scalar.activation(A_sb[:], es_ps, ACTF.Identity,
                             bias=linsc[:], scale=ise[:])
        f = p["fix"].tile([P, BPC], F32, tag=cx.tag("Fq"))
        nc.vector.tensor_tensor(f[:], hf[coh][:], A_sb[:], ALU.mult)
        Fq.append(f)
    out_ps = cx.ps_misc[0:1, 0:BPC]
    for coh in range(2):
        nc.tensor.matmul(out_ps, cx.ones_col.bitcast(F32), Fq[coh][:],
                         start=(coh == 0), stop=(coh == 1))
    lb = p["fix"].tile([1, 1], F32, tag="lb")
    nc.sync.dma_start(out=lb[:], in_=D["lin_b"][:, :])
    o_sb = p["fix"].tile([1, BPC], F32, tag="o_sb")
    nc.scalar.activation(o_sb[:], out_ps, ACTF.Identity, bias=lb[:], scale=1.0)
    nc.sync.dma_start(out=o_out[:, :], in_=o_sb[:])

    q_sb = p["fix"].tile([1, BPC], F32, tag="q_sb")
    nc.scalar.copy(q_sb[:], qd_ps[:])
    nc.sync.dma_start(out=o_quant[:, :], in_=q_sb[:])
    nc.sync.dma_start(out=o_counts[:, :], in_=c_acc[:])

    if o_dbg is not None:
        off = 0
        for t, w in ((h0c[0], 2048), (h0c[1], 2048), (h1c[0], 512), (h1c[1], 512),
                     (h2c[0], 512), (h2c[1], 512), (h3c[0], 512), (h3c[1], 512),
                     (hf[0], 8), (hf[1], 8)):
            nc.sync.dma_start(out=o_dbg[:, off:off + w], in_=t[:].bitcast(F32))
            off += w


# ----------------------------------------------------------------- host side

_NC_CACHE = {}


def _get_nc(dev=0, parts=4):
    key = (dev, parts)
    if key not in _NC_CACHE:
        _NC_CACHE[key] = build_module(dev, parts)
    return _NC_CACHE[key]


def _wt3x3(w):
    co, ci = w.shape[0], w.shape[1]
    return _r32r(np.ascontiguousarray(
        np.asarray(w).reshape(co, ci, 3, 3).transpose(2, 3, 1, 0).reshape(
            9 * ci, co)))


def prep_core_inputs(inputs, core):
    d = {}
    s = slice(core * BPC, (core + 1) * BPC)
    d["x8"] = _r32r(inputs["x"][s])
    w1b0 = np.asarray(inputs["b0_w1"]).reshape(256, 3, 3, 3).transpose(2, 3, 1, 0)
    w1b0 = w1b0.reshape(9, 3, 256)[TAP_ORDER].reshape(27, 256)
    d["w1t_b0"] = _r32r(np.ascontiguousarray(w1b0))
    d["w2t_b0"] = _wt3x3(inputs["b0_w2"])
    d["w1t_b1"] = _wt3x3(inputs["b1_w1"])
    d["w2t_b1"] = _wt3x3(inputs["b1_w2"])
    d["w1t_b2"] = _wt3x3(inputs["b2_w1"])
    d["w2t_b2"] = _wt3x3(inputs["b2_w2"])
    d["w1t_b3"] = _wt3x3(inputs["b3_w1"])
    d["w2t_b3"] = _wt3x3(inputs["b3_w2"])
    d["wsct_b0"] = _r32r(np.asarray(inputs["b0_wsc"]).reshape(256, 3).T)
    d["wsct_b1"] = _r32r(np.asarray(inputs["b1_wsc"]).reshape(256, 256).T)
    biases = np.stack([
        inputs["b0_b1"], inputs["b0_b2"], inputs["b0_bsc"],
        inputs["b1_b1"], inputs["b1_b2"], inputs["b1_bsc"],
        inputs["b2_b1"], inputs["b2_b2"],
        inputs["b3_b1"], inputs["b3_b2"]], axis=1).astype(np.float32)
    d["biases"] = np.ascontiguousarray(biases)
    for i in range(4):
        d[f"cb{i}"] = _r32r(inputs[f"cb{i}"])
    d["linT"] = _r32r(np.asarray(inputs["lin_w"]).reshape(1, 256).T)
    d["lin_b"] = np.asarray(inputs["lin_b"], np.float32).reshape(1, 1)
    d["embT"] = _r32r(np.asarray(inputs["emb_w"]).T)
    d["emb_nat"] = _r32r(inputs["emb_w"])
    d["y8"] = _r32r(np.asarray(inputs["y"][s], np.float32).reshape(1, BPC))
    d["iota100"] = np.arange(100, dtype=np.float32).reshape(100, 1)
    cc = np.zeros((P, 8), np.float32)
    cc[:, 0] = 1.0
    cc[:, 1] = C0
    cc[:, 2] = -2.0 * C0
    cc[:, 3] = C1
    cc[:, 4] = -2.0 * C1
    cc[0:64, 6] = -2.0 * C1
    cc[64:128, 7] = -2.0 * C1
    d["cc"] = _r32r(cc)
    d["ones_row_d"] = np.ones((1, P), np.float32)
    d["zpad"] = np.zeros((P, 2592), np.float32)
    return d


def run_cores(inputs, dev=0, **kw):
    nc = _get_nc(dev)
    in_maps = [prep_core_inputs(inputs, c) for c in range(NCORES)]
    return run_bass_kernel_spmd(nc, in_maps, core_ids=list(range(NCORES)), **kw)


def assemble(results):
    outs = np.concatenate([r["o_out"][0] for r in results]).reshape(64, 1)
    quant = np.concatenate([r["o_quant"][0] for r in results]).reshape(64, 1)
    counts = np.sum([r["o_counts"][0] for r in results], axis=0).astype(np.float32)
    probs = counts / np.float32(64 * 8 * 8)
    ppl = np.exp(-np.sum(probs * np.log(probs + np.float32(1e-10)),
                         dtype=np.float32)).astype(np.float32)
    return (outs.astype(np.float32), quant.astype(np.float32), ppl)


def kernel(**inputs):
    inputs = {k: np.asarray(v) for k, v in inputs.items()}
    last = None
    for attempt in range(4):
        try:
            res = run_cores(inputs)
            return assemble(res.results)
        except Exception as e:  # transient NRT_EXEC_UNIT_UNRECOVERABLE wedges
            last = e
            import time as _time
            _time.sleep(3.0)
            try:
                import jax
                jax.clear_caches()
                jax.extend.backend.clear_backends()
            except Exception:
                pass
            _time.sleep(2.0)
    raise last
